# revision 35
# baseline (speedup 1.0000x reference)
"""Trainium2 Bass kernel for a dense pre-norm transformer block.

Problem: B=2, N=2048, C=768, H=12 heads (D=64), MLP hidden 3072, f32 I/O.

Sharding (8 cores, no collectives): query-parallel. Core c handles batch
c//4 and query rows (c%4)*512 .. +512, for all heads. Each core computes
K/V for its full batch redundantly (4x) — cheaper than cross-core
collectives at these sizes. Each core's x is uploaded rolled so that its
own 512 query tokens are rows 0..511 (attention is permutation-invariant
over keys once the mask is rolled the same way), which keeps the device
program identical across cores.

v4 design notes:
- LN1 runs on the host; xn is uploaded directly in the feature-major
  (transposed) layout the projections consume, so the device starts
  matmulling ~immediately.
- Every DMA source is laid out on the host to be contiguous per
  partition; strided patterns made the descriptor generation (software
  dynamic DMA) take microseconds of engine time per transfer.
- The K/Q projections for head pair i+1 and the whole V projection are
  interleaved into earlier head pairs' attention streams as PE filler:
  attention alone leaves the PE at ~55% duty, which lets the HAM clock
  gate re-throttle the PE to 1.2 GHz and double every matmul.
- Softmax Z rows are collected in DRAM; two batched reciprocals (one
  overlapped under head pair 4-5, one in the tail) replace per-row
  reciprocals that would serialize the vector engine for 3.3us each.

Precision: bf16 matmul operands, f32 PSUM accumulation, f32 layernorm
stats and residuals. LN gains (g1/g2) and the attention 1/sqrt(D) scale
are folded into the weight matrices on the host. All LN/projection biases
in this problem are exactly zero (verified on host at call time).
"""

import os
import sys

for _p in ("/opt/trn_rl_repo",):
    if os.path.isdir(_p) and _p not in sys.path:
        sys.path.append(_p)

import numpy as np
import ml_dtypes

import concourse.bass as bass
import concourse.mybir as mybir
import concourse.tile as tile
from concourse.bass_utils import run_bass_kernel_spmd

# ---------------------------------------------------------------- constants
B, N, C = 2, 2048, 768
H, D = 12, 64
HID = 4 * C
SCALE = D ** -0.5
EPS = 1e-5
NCORES = 8
QS = N // 4          # queries per core = 512
QT = QS // 128       # query token tiles per core = 4
NT = N // 128        # token tiles per batch = 16
CT = C // 128        # feature tiles = 6
HT = HID // 128      # hidden tiles = 24

F32 = mybir.dt.float32
BF16 = mybir.dt.bfloat16
FP8 = mybir.dt.float8e4
DR = mybir.MatmulPerfMode.DoubleRow
AF = mybir.ActivationFunctionType
ALU = mybir.AluOpType

# fp8 scale folding: xn is uploaded x4 and the QKV weights x8 so almost no
# value lands in the e4m3 subnormal range; the x32 product scale is divided
# back out at the kT/qT evacuation and (for the V path) folded into Wp.
XSCALE = 4.0
WSCALE = 8.0
PSCALE = 1.0 / (XSCALE * WSCALE)
EXP_BIAS = -3.0   # keeps exp() outputs inside fp8 range; cancels via 1/Z


def _patch_tile_drain():
    """This walrus build rejects Drain instructions carrying >1 sem-wait
    ("Too many sync wait commands"). Split the TileContext exit-drain's
    waits across a chain of single-wait drains."""
    import concourse.tile as tile_mod

    if getattr(tile_mod.TileContext, "_ant_drain_patched", False):
        return

    def _patched(self, tick_clock, wait_clock):
        nc = self.nc
        drain_inst = nc.sync.drain()
        wait_clock.add_sem_waits(
            drain_inst.ins, tile_mod.ScopedClock({None: tick_clock.global_clock})
        )
        si = drain_inst.ins.sync_info
        if si is not None and si.on_wait and len(si.on_wait) > 1:
            extra = list(si.on_wait[1:])
            si.on_wait = [si.on_wait[0]]
            for w in extra:
                d2 = nc.sync.drain().ins
                si2 = d2.sync_info
                if si2 is None:
                    d2.sync_info = type(si)(on_wait=[w], on_update=[])
                else:
                    si2.on_wait = [w]
        nc.all_engine_barrier()
        assert self.sems is not None
        popped = nc._tile_sem_poison_stack.pop()
        assert popped is self._sem_poison
        nc.clear_and_free_semaphores(list(self.sems.allocated().values()))
        nc.all_engine_barrier()

    tile_mod.TileContext._drain_and_barrier = _patched
    tile_mod.TileContext._ant_drain_patched = True


_MAX_WAITS_BY_TYPE = {"InstDrain": 1, "InstDmaTransposeAnt": 1}
_DEFAULT_MAX_WAITS = 1


def _split_excess_waits(nc):
    """This walrus build rejects instructions carrying more than ~1 sem-wait
    ("Too many sync wait commands"). Move excess waits onto same-engine NOPs
    inserted immediately before the instruction."""
    nid = [0]

    def mk_nop(engine, wait):
        nid[0] += 1
        nop = mybir.InstNoOp(name=f"antw-{nid[0]}", ins=[], outs=[])
        nop.engine = engine
        nop.sync_info = mybir.SyncInfo(on_wait=[wait], on_update=[])
        return nop

    for bb in nc.main_func.blocks:
        new_list = []
        for ins in bb.instructions:
            si = ins.sync_info
            lim = _MAX_WAITS_BY_TYPE.get(type(ins).__name__, _DEFAULT_MAX_WAITS)
            if si is not None and si.on_wait and len(si.on_wait) > lim:
                extra = list(si.on_wait[lim:])
                si.on_wait = list(si.on_wait[:lim])
                for w in extra:
                    new_list.append(mk_nop(ins.engine, w))
            new_list.append(ins)
        bb.instructions[:] = new_list


def _layer_norm_tile(nc, pools, xt, rows=128):
    """LN stats for one (128, C) f32 tile -> (mu, rstd) per-partition aps."""
    spool = pools["stats"]
    stats = spool.tile([128, 3, 6], F32, tag="stats", name="stats")
    for sg in range(3):
        nc.vector.bn_stats(
            out=stats[:rows, sg, :], in_=xt[:rows, sg * 256:(sg + 1) * 256]
        )
    mv = spool.tile([128, 2], F32, tag="mv", name="mv")
    nc.vector.bn_aggr(out=mv[:rows], in_=stats[:rows])
    rstd = spool.tile([128, 1], F32, tag="rstd", name="rstd")
    nc.scalar.activation(
        out=rstd[:rows], in_=mv[:rows, 1:2], func=AF.Sqrt, bias=pools["eps"][:rows]
    )
    rstd2 = spool.tile([128, 1], F32, tag="rstd2", name="rstd2")
    nc.vector.reciprocal(out=rstd2[:rows], in_=rstd[:rows])
    return mv[:rows, 0:1], rstd2[:rows]


def build_program():
    """Build the SPMD single-core program (same BIR for all 8 cores)."""
    _patch_tile_drain()
    nc = bass.Bass()

    # Host-side layouts are exactly the SBUF layouts (contiguous per
    # partition) so every transfer is a fast hardware-dynamic DMA.
    xnt = nc.declare_dram_parameter("xnt", [128, 4 * CT * QS], FP8, isOutput=False)
    xm = nc.declare_dram_parameter("xm", [QS, C], F32, isOutput=False)
    mm = nc.declare_dram_parameter("mm", [128, NT * QS], BF16, isOutput=False)
    wqt = nc.declare_dram_parameter("wqt", [128, CT * CT * 128], FP8, isOutput=False)
    wkt = nc.declare_dram_parameter("wkt", [128, CT * CT * 128], FP8, isOutput=False)
    wvt = nc.declare_dram_parameter("wvt", [128, CT * C], FP8, isOutput=False)
    wpt = nc.declare_dram_parameter("wpt", [128, CT * C], BF16, isOutput=False)
    w1t = nc.declare_dram_parameter("w1t", [128, HT * CT * 128], BF16, isOutput=False)
    w2t = nc.declare_dram_parameter("w2t", [128, HT * C], BF16, isOutput=False)
    idn = nc.declare_dram_parameter("idn", [128, 128], BF16, isOutput=False)
    out = nc.declare_dram_parameter("out", [QS, C], F32, isOutput=True)

    with tile.TileContext(nc) as tc:
        _build_body(nc, tc, xnt, xm, mm, wqt, wkt, wvt, wpt, w1t, w2t, idn, out)
    _split_excess_waits(nc)
    return nc


def _transpose_128x768(nc, pst_pool, ident, src_bf16, dst, dst_tslice):
    """PE-transpose a (128, 768) bf16 tile into dst[:, 0:CT, dst_tslice]."""
    pst = pst_pool.tile([128, C], BF16, tag="pst", name="pst")
    for dt in range(CT):
        nc.tensor.transpose(
            pst[:, dt * 128:(dt + 1) * 128],
            src_bf16[:, dt * 128:(dt + 1) * 128],
            ident[:],
        )
    nc.scalar.copy(
        out=dst[:, :, dst_tslice],
        in_=pst.rearrange("p (dt q) -> p dt q", dt=CT),
    )


def _build_body(nc, tc, xnt, xm, mm, wqt, wkt, wvt, wpt, w1t, w2t, idn, out):
    from contextlib import ExitStack

    ctx = ExitStack()
    with ctx:
        # ---------------- pools that live to the end of the kernel
        const_p = ctx.enter_context(tc.tile_pool(name="const", bufs=1))
        xmt_p = ctx.enter_context(tc.tile_pool(name="xmtp", bufs=1))
        stats_p = ctx.enter_context(tc.tile_pool(name="statsp", bufs=4))
        ps_p = ctx.enter_context(tc.tile_pool(name="psp", bufs=2, space="PSUM"))

        eps_t = const_p.tile([128, 1], F32, name="eps_t")
        nc.vector.memset(eps_t[:], EPS)
        ident = const_p.tile([128, 128], BF16, name="ident")
        nc.sync.dma_start(out=ident[:], in_=idn[:])
        pools = {"stats": stats_p, "eps": eps_t, "ident": ident}

        xmt = [xmt_p.tile([128, C], F32, tag=f"xmt{i}", name=f"xmt{i}")
               for i in range(QT)]

        # ---------------- pools that live through attention + proj
        oT_p = ctx.enter_context(tc.tile_pool(name="oTp", bufs=1))
        wp_p = ctx.enter_context(tc.tile_pool(name="wpp", bufs=1))
        oTu = oT_p.tile([128, CT, QS], BF16, name="oTu")   # unnormalized
        oT = oT_p.tile([128, CT, QS], BF16, name="oT")     # normalized
        wp_sb = wp_p.tile([128, CT, C], BF16, name="wp_sb")
        cps_ctx = ctx.enter_context(ExitStack())
        pso_p = cps_ctx.enter_context(
            tc.tile_pool(name="psop", bufs=2, space="PSUM"))
        ps2_p = cps_ctx.enter_context(
            tc.tile_pool(name="ps2p", bufs=2, space="PSUM"))

        # ---------------- pools for K/V/Q + attention (released after C)
        kvq_ctx = ctx.enter_context(ExitStack())
        kT_p = kvq_ctx.enter_context(tc.tile_pool(name="kTp", bufs=1))
        v_p = kvq_ctx.enter_context(tc.tile_pool(name="vp", bufs=1))
        qT_p = kvq_ctx.enter_context(tc.tile_pool(name="qTp", bufs=1))
        mm_p = kvq_ctx.enter_context(tc.tile_pool(name="mmp", bufs=1))
        pc_p = kvq_ctx.enter_context(tc.tile_pool(name="pcp", bufs=13))
        z_p = kvq_ctx.enter_context(tc.tile_pool(name="zp", bufs=2))
        zb_p = kvq_ctx.enter_context(tc.tile_pool(name="zbp", bufs=2))
        zd_p = kvq_ctx.enter_context(
            tc.tile_pool(name="zdp", bufs=1, space="DRAM"))
        # xnT freed once the V projection has consumed it (mid-attention);
        # created last so it can be popped first (pools release LIFO).
        xnT_ctx = kvq_ctx.enter_context(ExitStack())
        xnT_p = xnT_ctx.enter_context(tc.tile_pool(name="xnTp", bufs=1))
        wqk_p = xnT_ctx.enter_context(tc.tile_pool(name="wqkp", bufs=1))

        VP = 68   # vaug inner stride: D + ones-col + pad to a 16B multiple
        xnT = xnT_p.tile([128, 4, CT, QS], FP8, name="xnT")
        kT = kT_p.tile([128, CT, N], BF16, name="kT")
        vaug = v_p.tile([128, NT, H, VP], FP8, name="vaug")
        qT = qT_p.tile([128, CT, QS], BF16, name="qT")
        mmsb = mm_p.tile([128, NT, QS], BF16, name="mmsb")

        nc.vector.memset(vaug[:, :, :, D:D + 1], 1.0)
        nc.vector.memset(vaug[:, :, :, D + 1:VP], 0.0)
        ebias = const_p.tile([128, 1], F32, name="ebias")
        nc.vector.memset(ebias[:], EXP_BIAS)

        # ---------------- input + weight DMAs (contiguous, multi-queue)
        xntr = xnt.rearrange("p (c kc t) -> p c kc t", c=4, kc=CT)
        for tch in range(4):
            nc.sync.dma_start(out=xnT[:, tch], in_=xntr[:, tch])
        # wk/wq are uploaded per-head-pair (dt-major) so head pair 0's
        # projection can start after a small transfer.
        wkr = wkt.rearrange("p (dt kc q) -> p dt kc q", dt=CT, kc=CT)
        wqr = wqt.rearrange("p (dt kc q) -> p dt kc q", dt=CT, kc=CT)
        wk_sb = wqk_p.tile([128, CT, CT, 128], FP8, tag="wk", name="wk_sb")
        wq_sb = wqk_p.tile([128, CT, CT, 128], FP8, tag="wq", name="wq_sb")
        for dt in range(CT):
            nc.gpsimd.dma_start(out=wk_sb[:, dt], in_=wkr[:, dt])
            nc.gpsimd.dma_start(out=wq_sb[:, dt], in_=wqr[:, dt])
        nc.scalar.dma_start(
            out=mmsb[:], in_=mm.rearrange("p (kc q) -> p kc q", kc=NT)
        )
        wv_sb = wqk_p.tile([128, CT, C], FP8, tag="wv", name="wv_sb")
        nc.gpsimd.dma_start(
            out=wv_sb[:], in_=wvt.rearrange("p (kc d) -> p kc d", kc=CT)
        )
        for t in range(QT):
            nc.sync.dma_start(out=xmt[t][:], in_=xm[t * 128:(t + 1) * 128, :])
        nc.scalar.dma_start(
            out=wp_sb[:], in_=wpt.rearrange("p (kc d) -> p kc d", kc=CT)
        )

        # ---------------- filler units: each emits ~1-2us of independent PE
        # work (plus its PSUM->SBUF evacuation) to keep the PE array dense
        # (and therefore at full clock) inside ACT-bound attention phases.
        def k_unit(dt, tch):
            ps = ps_p.tile([128, 512], F32, tag="ps", name="ps")
            for k2 in range(CT // 2):
                nc.tensor.matmul(
                    ps[:],
                    wk_sb[:, dt, 2 * k2:2 * k2 + 2, :],
                    xnT[:, tch, 2 * k2:2 * k2 + 2, :],
                    start=(k2 == 0), stop=(k2 == CT // 2 - 1),
                    skip_group_check=True, perf_mode=DR,
                )
            nc.vector.tensor_scalar(
                out=kT[:, dt, tch * 512:(tch + 1) * 512], in0=ps[:],
                scalar1=float(PSCALE), scalar2=None, op0=ALU.mult,
            )

        def q_unit(dt):
            ps = ps_p.tile([128, 512], F32, tag="ps", name="ps")
            for k2 in range(CT // 2):
                nc.tensor.matmul(
                    ps[:],
                    wq_sb[:, dt, 2 * k2:2 * k2 + 2, :],
                    xnT[:, 0, 2 * k2:2 * k2 + 2, :],
                    start=(k2 == 0), stop=(k2 == CT // 2 - 1),
                    skip_group_check=True, perf_mode=DR,
                )
            nc.vector.tensor_scalar(
                out=qT[:, dt, :], in0=ps[:],
                scalar1=float(PSCALE), scalar2=None, op0=ALU.mult,
            )

        def v_unit(tt, nch):
            tch, sub = divmod(tt, 4)
            ps = ps_p.tile([128, 384], F32, tag="ps", name="psv")
            for k2 in range(CT // 2):
                nc.tensor.matmul(
                    ps[:],
                    xnT[:, tch, 2 * k2:2 * k2 + 2, sub * 128:(sub + 1) * 128],
                    wv_sb[:, 2 * k2:2 * k2 + 2, nch * 384:(nch + 1) * 384],
                    start=(k2 == 0), stop=(k2 == CT // 2 - 1),
                    skip_group_check=True, perf_mode=DR,
                )
            # v stays at x32 scale inside fp8 vaug (better fp8 resolution);
            # Wp carries the 1/32 on the host.
            nc.vector.tensor_copy(
                out=vaug[:, tt, nch * 6:(nch + 1) * 6, 0:D],
                in_=ps.rearrange("p (h d) -> p h d", h=6),
            )

        # per-head-pair filler rations: during hp i, emit exactly head pair
        # i+1's five K/Q units (one every ~3 steps); hp 0 additionally
        # carries the whole V projection (2 units per step).
        fillers = {}
        for dt in range(1, CT):
            units = [lambda dt=dt, tch=tch: k_unit(dt, tch) for tch in range(4)]
            units.append(lambda dt=dt: q_unit(dt))
            fillers[dt - 1] = units
        v_work = [(tt, nch) for tt in range(NT) for nch in range(2)]

        # head pair 0's K/Q run up front (attention depends on them)
        for tch in range(4):
            k_unit(0, tch)
        q_unit(0)

        # Z bookkeeping: Z rows (PSUM row 64 of each AV accumulator) are
        # copied to DRAM as they appear; two batched reciprocals produce
        # 1/Z spread over 128 partitions... no — batched over head rows,
        # overlapped under later head pairs' compute.
        zdA = zd_p.tile([H, QS], F32, name="zdA", tag="zdA")
        zdR = zd_p.tile([H, QS], F32, name="zdR", tag="zdR")

        def z_batch(h0, h1):
            """Emit fine-grained work items that turn Z rows h0..h1-1 (in
            DRAM) into 1/Z rows: one DMA in, reciprocal in 4 column chunks
            (so no single DVE op exceeds ~1us), one DMA out."""
            nrow = h1 - h0
            zsb = z_p.tile([H, QS], F32, tag="zsb", name="zsb")
            zrb = z_p.tile([H, QS], F32, tag="zrb", name="zrb")
            items = [lambda: nc.gpsimd.dma_start(
                out=zsb[0:nrow, :], in_=zdA[h0:h1, :])]
            for cc in range(4):
                items.append(lambda cc=cc: nc.vector.reciprocal(
                    out=zrb[0:nrow, cc * 128:(cc + 1) * 128],
                    in_=zsb[0:nrow, cc * 128:(cc + 1) * 128],
                ))
            items.append(lambda: nc.gpsimd.dma_start(
                out=zdR[h0:h1, :], in_=zrb[0:nrow, :]))
            return items

        def z_apply(hp):
            """Broadcast 1/Z for head pair hp and normalize oTu -> oT.
            The multiply runs on the (otherwise idle) gpsimd engine."""
            zbig = zb_p.tile([128, 512], F32, tag="zbig", name="zbig")
            for half in range(2):
                nc.gpsimd.dma_start(
                    out=zbig[half * 64:(half + 1) * 64, :],
                    in_=zdR[hp * 2 + half:hp * 2 + half + 1, :]
                    .to_broadcast([64, 512]),
                )
            nc.gpsimd.tensor_tensor(
                out=oT[:, hp, :], in0=oTu[:, hp, :], in1=zbig[:, :],
                op=ALU.mult,
            )

        # ---------------- phase C: attention
        AV_LAG = 5
        # Z normalization batches, emitted as small work items one per
        # attention step: head pairs 0..3 resolve under hp 4, hp 4 under
        # hp 5; only hp 5's own Z (plus proj) remains for the tail.
        post_work = {
            4: z_batch(0, 8) + [lambda h=h: z_apply(h) for h in range(4)],
            5: z_batch(8, 10) + [lambda: z_apply(4)],
        }
        for hp in range(CT):
            psos = [
                pso_p.tile([VP, 512], F32, tag="pso", name="pso"),
                pso_p.tile([VP, 512], F32, tag="pso", name="pso"),
            ]

            def emit_av(half, pc, kc2):
                # one DoubleRow matmul covers both 128-key chunks in pc
                nc.tensor.matmul(
                    psos[half][:],
                    vaug[:, 2 * kc2:2 * kc2 + 2, hp * 2 + half, :],
                    pc[:],
                    start=(kc2 == 0), stop=(kc2 == NT // 2 - 1),
                    skip_group_check=True, perf_mode=DR,
                )

            pend = {0: [], 1: []}
            steps_done = 0
            for kc2 in range(NT // 2):
                for half in range(2):
                    p0 = half * 64
                    pss = ps2_p.tile([128, 1024], F32, tag="pss", name="pss")
                    for j in range(2):
                        kc = kc2 * 2 + j
                        nc.tensor.matmul(
                            pss[:, j * 512:(j + 1) * 512],
                            kT[p0:p0 + 64, hp, kc * 128:(kc + 1) * 128],
                            qT[p0:p0 + 64, hp, :],
                            start=True, stop=True,
                        )
                    pcb = pc_p.tile([128, 2, QS], BF16, tag="pcb", name="pcb",
                                    bufs=4)
                    nc.scalar.activation(
                        out=pcb[:],
                        in_=pss.rearrange("p (two q) -> p two q", two=2),
                        func=AF.Exp, bias=ebias[:],
                    )
                    pc = pc_p.tile([128, 2, QS], FP8, tag="pc", name="pc")
                    nc.vector.tensor_mul(
                        pc[:], pcb[:], mmsb[:, kc2 * 2:kc2 * 2 + 2, :]
                    )
                    pend[half].append((pc, kc2))
                    if len(pend[half]) > AV_LAG:
                        pcq, k2q = pend[half].pop(0)
                        emit_av(half, pcq, k2q)
                    # PE fillers: V projection inside head pair 0 (its AV
                    # chunks consume vaug progressively), K/Q projections
                    # for head pair hp+1 spread across hp's steps.
                    steps_done += 1
                    if hp == 0:
                        for _ in range(2):
                            if v_work:
                                tt, nch = v_work.pop(0)
                                v_unit(tt, nch)
                    ration = fillers.get(hp, [])
                    if steps_done % 3 == 0 and ration:
                        ration.pop(0)()
                    pw = post_work.get(hp, [])
                    if pw:
                        pw.pop(0)()
            for half in range(2):
                for pcq, k2q in pend[half]:
                    emit_av(half, pcq, k2q)
            # head pair epilogue: evacuate o (unnormalized) and Z
            for half in range(2):
                nc.vector.tensor_copy(
                    out=oTu[half * 64:(half + 1) * 64, hp, :],
                    in_=psos[half][0:64, :],
                )
                zs = z_p.tile([65, 512], F32, tag="zs", name="zs")
                nc.vector.tensor_copy(out=zs[64:65, :], in_=psos[half][64:65, :])
                nc.gpsimd.dma_start(
                    out=zdA[hp * 2 + half:hp * 2 + half + 1, :],
                    in_=zs[64:65, :],
                )
            if hp == 0:
                xnT_ctx.close()

        # tail: only head pair 5's Z remains
        for item in z_batch(10, 12):
            item()
        z_apply(5)

        # ---------------- phase D: proj + residual + LN2 -> xn2T
        kvq_ctx.close()
        cps_ctx.close()
        pst_p = ctx.enter_context(tc.tile_pool(name="pstp", bufs=2, space="PSUM"))
        x1_p = ctx.enter_context(tc.tile_pool(name="x1p", bufs=1))
        xn2T_p = ctx.enter_context(tc.tile_pool(name="xn2Tp", bufs=1))
        w2_p = ctx.enter_context(tc.tile_pool(name="w2p", bufs=1))
        x1t = [x1_p.tile([128, C], F32, tag=f"x1t{i}", name=f"x1t{i}")
               for i in range(QT)]
        xn2T = xn2T_p.tile([128, CT, QS], BF16, name="xn2T")
        w2_sb = w2_p.tile([128, HT, C], BF16, name="w2_sb")
        w2r = w2t.rearrange("p (ht c) -> p ht c", ht=HT)
        for h in range(3):
            nc.gpsimd.dma_start(
                out=w2_sb[:, h * 8:(h + 1) * 8, :], in_=w2r[:, h * 8:(h + 1) * 8, :]
            )
        with tc.tile_pool(name="xn2", bufs=2) as xn2_p:
            for tt in range(QT):
                for nch in range(2):
                    ps = ps_p.tile([128, 384], F32, tag="ps", name="ps")
                    for kc in range(CT):
                        nc.tensor.matmul(
                            ps[:],
                            oT[:, kc, tt * 128:(tt + 1) * 128],
                            wp_sb[:, kc, nch * 384:(nch + 1) * 384],
                            start=(kc == 0), stop=(kc == CT - 1),
                        )
                    nc.vector.scalar_tensor_tensor(
                        out=x1t[tt][:, nch * 384:(nch + 1) * 384],
                        in0=ps[:], scalar=1.0,
                        in1=xmt[tt][:, nch * 384:(nch + 1) * 384],
                        op0=ALU.mult, op1=ALU.add,
                    )
                mu, rstd = _layer_norm_tile(nc, pools, x1t[tt])
                xn2 = xn2_p.tile([128, C], BF16, tag="xn2", name="xn2")
                nc.vector.tensor_scalar(
                    out=xn2[:], in0=x1t[tt][:], scalar1=mu, scalar2=rstd,
                    op0=ALU.subtract, op1=ALU.mult,
                )
                _transpose_128x768(
                    nc, pst_p, ident, xn2, xn2T, slice(tt * 128, (tt + 1) * 128)
                )

        # ---------------- phase E: MLP. fc2 accumulation for the first two
        # token tiles rides along inside the fc1 loop so the PE never waits
        # for the full gelu sweep.
        with tc.tile_pool(name="gTp", bufs=1) as gT_p, \
             tc.tile_pool(name="w1p", bufs=4) as w1_p, \
             tc.tile_pool(name="psE", bufs=4, space="PSUM") as psE_p, \
             tc.tile_pool(name="op", bufs=2) as o_p:
            gT = gT_p.tile([128, HT, QS], BF16, name="gT")
            w1r = w1t.rearrange("p (ht kc q) -> p ht kc q", ht=HT, kc=CT)
            NEARLY = 2
            chains = {}
            for tt in range(NEARLY):
                for nch in range(2):
                    chains[(tt, nch)] = psE_p.tile(
                        [128, 384], F32, tag="psE", name="psE"
                    )
            for ht in range(HT):
                w1c = w1_p.tile([128, CT, 128], BF16, tag="w1c", name="w1c")
                nc.sync.dma_start(out=w1c[:], in_=w1r[:, ht])
                ps = ps_p.tile([128, 512], F32, tag="ps", name="ps")
                for kc in range(CT):
                    nc.tensor.matmul(
                        ps[:],
                        w1c[:, kc, :],
                        xn2T[:, kc, :],
                        start=(kc == 0), stop=(kc == CT - 1),
                    )
                nc.scalar.activation(out=gT[:, ht, :], in_=ps[:], func=AF.Gelu)
                for tt in range(NEARLY):
                    for nch in range(2):
                        nc.tensor.matmul(
                            chains[(tt, nch)][:],
                            gT[:, ht, tt * 128:(tt + 1) * 128],
                            w2_sb[:, ht, nch * 384:(nch + 1) * 384],
                            start=(ht == 0), stop=(ht == HT - 1),
                            skip_group_check=True,
                        )
            for tt in range(QT):
                outt = o_p.tile([128, C], F32, tag="outt", name="outt")
                for nch in range(2):
                    if tt < NEARLY:
                        ps2 = chains[(tt, nch)]
                    else:
                        ps2 = psE_p.tile([128, 384], F32, tag="psE", name="psE")
                        for ht in range(HT):
                            nc.tensor.matmul(
                                ps2[:],
                                gT[:, ht, tt * 128:(tt + 1) * 128],
                                w2_sb[:, ht, nch * 384:(nch + 1) * 384],
                                start=(ht == 0), stop=(ht == HT - 1),
                            )
                    nc.vector.scalar_tensor_tensor(
                        out=outt[:, nch * 384:(nch + 1) * 384],
                        in0=ps2[:], scalar=1.0,
                        in1=x1t[tt][:, nch * 384:(nch + 1) * 384],
                        op0=ALU.mult, op1=ALU.add,
                    )
                # spread output DMAs across queues so the final transfers
                # overlap instead of serializing on one queue
                eng = [nc.sync, nc.gpsimd, nc.scalar, nc.sync][tt]
                eng.dma_start(
                    out=out[tt * 128:(tt + 1) * 128, :], in_=outt[:]
                )


# ---------------------------------------------------------------- host side
_CACHED_NC = None


def _get_nc():
    global _CACHED_NC
    if _CACHED_NC is None:
        _CACHED_NC = build_program()
    return _CACHED_NC


def _part_major(a, inner_shape):
    """(CT*128, X) row-major -> (128, prod(inner_shape)) where the leading
    dim is split (blk, 128) and partitions become major: out[p, blk, :] =
    a[blk*128 + p, :]."""
    nblk = a.shape[0] // 128
    return np.ascontiguousarray(
        a.reshape((nblk, 128) + a.shape[1:]).swapaxes(0, 1).reshape(128, -1)
    )


def make_in_maps(x, mask, g1, b1, Wq, Wkv, Wp, bp, g2, b2, W1, bf1, W2, bf2):
    f32 = np.float32
    bf = ml_dtypes.bfloat16
    x = np.asarray(x, f32)
    mask = np.asarray(mask, f32)
    g1 = np.asarray(g1, f32); b1 = np.asarray(b1, f32)
    g2 = np.asarray(g2, f32); b2 = np.asarray(b2, f32)
    Wq = np.asarray(Wq, f32); Wkv = np.asarray(Wkv, f32); Wp = np.asarray(Wp, f32)
    W1 = np.asarray(W1, f32); W2 = np.asarray(W2, f32)
    bp = np.asarray(bp, f32); bf1 = np.asarray(bf1, f32); bf2 = np.asarray(bf2, f32)

    Wk, Wv = Wkv[:C], Wkv[C:]
    # fold LN gains + attention scale into the weights; biases must be zero
    # (they are, for this problem's setup_inputs) for this fast path.
    zero_rows = [
        (b1 @ Wq.T) * SCALE, b1 @ Wk.T, b1 @ Wv.T, bp,
        bf1 + b2 @ W1.T, bf2,
    ]
    for r in zero_rows:
        assert np.abs(r).max() == 0.0, "nonzero bias path not implemented"

    # device layouts ------------------------------------------------------
    f8 = ml_dtypes.float8_e4m3fn
    # wk/wq: [128, dt, kc, 128] with wk[p, dt, kc, q] = WkT[kc*128+p, dt*128+q]
    # QKV weights are uploaded fp8 at x8 scale (see XSCALE/WSCALE notes).
    wkT = (Wk * g1[None, :]).T * WSCALE   # (C in-feat, C out-feat)
    wqT = (Wq * g1[None, :] * SCALE).T * WSCALE
    wk_h = _part_major(wkT, None).reshape(128, CT, CT, 128)   # p, kc, dt, q
    wq_h = _part_major(wqT, None).reshape(128, CT, CT, 128)
    wk_h = np.ascontiguousarray(wk_h.swapaxes(1, 2)).reshape(128, -1).astype(f8)
    wq_h = np.ascontiguousarray(wq_h.swapaxes(1, 2)).reshape(128, -1).astype(f8)
    wv_h = _part_major((Wv * g1[None, :]).T * WSCALE, None).astype(f8)
    # v arrives at x(XSCALE*WSCALE) scale; fold the inverse into Wp
    wp_h = _part_major(Wp.T * PSCALE, None).astype(bf)
    # w1: [128, ht, kc, 128]: w1[p, ht, kc, q] = W1T[kc*128+p, ht*128+q]
    w1T = (W1 * g2[None, :]).T            # (C, HID)
    w1_h = _part_major(w1T, None).reshape(128, CT, HT, 128)
    w1_h = np.ascontiguousarray(w1_h.swapaxes(1, 2)).reshape(128, -1).astype(bf)
    w2_h = _part_major(W2.T, None).astype(bf)                  # p,(ht c)
    idn_h = np.eye(128, dtype=bf)

    # host-side LN1 (plain: gains/biases are folded into the weights above)
    mu = x.mean(axis=-1, keepdims=True)
    var = x.var(axis=-1, keepdims=True)
    xn_full = (x - mu) / np.sqrt(var + EPS)

    in_maps = []
    for c in range(NCORES):
        b, qi = divmod(c, 4)
        q0 = qi * QS
        xr = np.roll(x[b], -q0, axis=0)                    # my tokens first
        xnr = np.roll(xn_full[b], -q0, axis=0)
        # xnt: chunk-major feature-major: [128, tch, kc, 512]
        # xnt[p, tch, kc, t] = XSCALE * xn[tch*512 + t, kc*128 + p]
        xnt_h = np.ascontiguousarray(
            (xnr.T * XSCALE)
            .reshape(CT, 128, 4, QS).transpose(1, 2, 0, 3).reshape(128, -1)
        ).astype(f8)
        km = np.roll(1.0 - mask[b].T, -q0, axis=0)         # keys rolled too
        mmc = _part_major(
            np.ascontiguousarray(km[:, q0:q0 + QS]), None
        ).astype(bf)
        in_maps.append({
            "xnt": xnt_h,
            "xm": np.ascontiguousarray(xr[:QS]),
            "mm": mmc,
            "wqt": wq_h, "wkt": wk_h, "wvt": wv_h, "wpt": wp_h,
            "w1t": w1_h, "w2t": w2_h, "idn": idn_h,
        })
    return in_maps


def kernel(**inputs):
    nc = _get_nc()
    in_maps = make_in_maps(**inputs)
    res = run_bass_kernel_spmd(nc, in_maps, core_ids=list(range(NCORES)))
    out = np.empty((B, N, C), np.float32)
    for c in range(NCORES):
        b, qi = divmod(c, 4)
        q0 = qi * QS
        out[b, q0:q0 + QS] = res.results[c]["out"]
    return out


if __name__ == "__main__":
    print("building program...")
    nc = _get_nc()
    print("instructions:", sum(len(bb.instructions) for bb in nc.main_func.blocks))


# revision 36
# speedup vs baseline: 1.0702x; 1.0702x over previous
"""Trainium2 Bass kernel for a dense pre-norm transformer block.

Problem: B=2, N=2048, C=768, H=12 heads (D=64), MLP hidden 3072, f32 I/O.

Sharding (8 cores, no collectives): query-parallel. Core c handles batch
c//4 and query rows (c%4)*512 .. +512, for all heads. Each core computes
K/V for its full batch redundantly (4x) — cheaper than cross-core
collectives at these sizes. Each core's x is uploaded rolled so that its
own 512 query tokens are rows 0..511 (attention is permutation-invariant
over keys once the mask is rolled the same way), which keeps the device
program identical across cores.

v4 design notes:
- LN1 runs on the host; xn is uploaded directly in the feature-major
  (transposed) layout the projections consume, so the device starts
  matmulling ~immediately.
- Every DMA source is laid out on the host to be contiguous per
  partition; strided patterns made the descriptor generation (software
  dynamic DMA) take microseconds of engine time per transfer.
- The K/Q projections for head pair i+1 and the whole V projection are
  interleaved into earlier head pairs' attention streams as PE filler:
  attention alone leaves the PE at ~55% duty, which lets the HAM clock
  gate re-throttle the PE to 1.2 GHz and double every matmul.
- Softmax Z rows are collected in DRAM; two batched reciprocals (one
  overlapped under head pair 4-5, one in the tail) replace per-row
  reciprocals that would serialize the vector engine for 3.3us each.

Precision: bf16 matmul operands, f32 PSUM accumulation, f32 layernorm
stats and residuals. LN gains (g1/g2) and the attention 1/sqrt(D) scale
are folded into the weight matrices on the host. All LN/projection biases
in this problem are exactly zero (verified on host at call time).
"""

import os
import sys

for _p in ("/opt/trn_rl_repo",):
    if os.path.isdir(_p) and _p not in sys.path:
        sys.path.append(_p)

import numpy as np
import ml_dtypes

import concourse.bass as bass
import concourse.mybir as mybir
import concourse.tile as tile
from concourse.bass_utils import run_bass_kernel_spmd

# ---------------------------------------------------------------- constants
B, N, C = 2, 2048, 768
H, D = 12, 64
HID = 4 * C
SCALE = D ** -0.5
EPS = 1e-5
NCORES = 8
QS = N // 4          # queries per core = 512
QT = QS // 128       # query token tiles per core = 4
NT = N // 128        # token tiles per batch = 16
CT = C // 128        # feature tiles = 6
HT = HID // 128      # hidden tiles = 24

F32 = mybir.dt.float32
BF16 = mybir.dt.bfloat16
FP8 = mybir.dt.float8e4
DR = mybir.MatmulPerfMode.DoubleRow
AF = mybir.ActivationFunctionType
ALU = mybir.AluOpType

# fp8 scale folding: xn is uploaded x4 and the QKV weights x8 so almost no
# value lands in the e4m3 subnormal range; the x32 product scale is divided
# back out at the kT/qT evacuation and (for the V path) folded into Wp.
XSCALE = 4.0
WSCALE = 8.0
PSCALE = 1.0 / (XSCALE * WSCALE)
EXP_BIAS = -3.0   # keeps exp() outputs inside fp8 range; cancels via 1/Z


def _patch_tile_drain():
    """This walrus build rejects Drain instructions carrying >1 sem-wait
    ("Too many sync wait commands"). Split the TileContext exit-drain's
    waits across a chain of single-wait drains."""
    import concourse.tile as tile_mod

    if getattr(tile_mod.TileContext, "_ant_drain_patched", False):
        return

    def _patched(self, tick_clock, wait_clock):
        nc = self.nc
        drain_inst = nc.sync.drain()
        wait_clock.add_sem_waits(
            drain_inst.ins, tile_mod.ScopedClock({None: tick_clock.global_clock})
        )
        si = drain_inst.ins.sync_info
        if si is not None and si.on_wait and len(si.on_wait) > 1:
            extra = list(si.on_wait[1:])
            si.on_wait = [si.on_wait[0]]
            for w in extra:
                d2 = nc.sync.drain().ins
                si2 = d2.sync_info
                if si2 is None:
                    d2.sync_info = type(si)(on_wait=[w], on_update=[])
                else:
                    si2.on_wait = [w]
        nc.all_engine_barrier()
        assert self.sems is not None
        popped = nc._tile_sem_poison_stack.pop()
        assert popped is self._sem_poison
        nc.clear_and_free_semaphores(list(self.sems.allocated().values()))
        nc.all_engine_barrier()

    tile_mod.TileContext._drain_and_barrier = _patched
    tile_mod.TileContext._ant_drain_patched = True


_MAX_WAITS_BY_TYPE = {"InstDrain": 1, "InstDmaTransposeAnt": 1}
_DEFAULT_MAX_WAITS = 1


def _split_excess_waits(nc):
    """This walrus build rejects instructions carrying more than ~1 sem-wait
    ("Too many sync wait commands"). Move excess waits onto same-engine NOPs
    inserted immediately before the instruction."""
    nid = [0]

    def mk_nop(engine, wait):
        nid[0] += 1
        nop = mybir.InstNoOp(name=f"antw-{nid[0]}", ins=[], outs=[])
        nop.engine = engine
        nop.sync_info = mybir.SyncInfo(on_wait=[wait], on_update=[])
        return nop

    for bb in nc.main_func.blocks:
        new_list = []
        for ins in bb.instructions:
            si = ins.sync_info
            lim = _MAX_WAITS_BY_TYPE.get(type(ins).__name__, _DEFAULT_MAX_WAITS)
            if si is not None and si.on_wait and len(si.on_wait) > lim:
                extra = list(si.on_wait[lim:])
                si.on_wait = list(si.on_wait[:lim])
                for w in extra:
                    new_list.append(mk_nop(ins.engine, w))
            new_list.append(ins)
        bb.instructions[:] = new_list


def _layer_norm_tile(nc, pools, xt, rows=128):
    """LN stats for one (128, C) f32 tile -> (mu, rstd) per-partition aps."""
    spool = pools["stats"]
    stats = spool.tile([128, 3, 6], F32, tag="stats", name="stats")
    for sg in range(3):
        nc.vector.bn_stats(
            out=stats[:rows, sg, :], in_=xt[:rows, sg * 256:(sg + 1) * 256]
        )
    mv = spool.tile([128, 2], F32, tag="mv", name="mv")
    nc.vector.bn_aggr(out=mv[:rows], in_=stats[:rows])
    rstd = spool.tile([128, 1], F32, tag="rstd", name="rstd")
    nc.scalar.activation(
        out=rstd[:rows], in_=mv[:rows, 1:2], func=AF.Sqrt, bias=pools["eps"][:rows]
    )
    rstd2 = spool.tile([128, 1], F32, tag="rstd2", name="rstd2")
    nc.vector.reciprocal(out=rstd2[:rows], in_=rstd[:rows])
    return mv[:rows, 0:1], rstd2[:rows]


def build_program():
    """Build the SPMD single-core program (same BIR for all 8 cores)."""
    _patch_tile_drain()
    nc = bass.Bass()

    # Host-side layouts are exactly the SBUF layouts (contiguous per
    # partition) so every transfer is a fast hardware-dynamic DMA.
    xnt = nc.declare_dram_parameter("xnt", [128, 4 * CT * QS], FP8, isOutput=False)
    xm = nc.declare_dram_parameter("xm", [QS, C], F32, isOutput=False)
    mm = nc.declare_dram_parameter("mm", [128, NT * QS], BF16, isOutput=False)
    wqt = nc.declare_dram_parameter("wqt", [128, CT * CT * 128], FP8, isOutput=False)
    wkt = nc.declare_dram_parameter("wkt", [128, CT * CT * 128], FP8, isOutput=False)
    wvt = nc.declare_dram_parameter("wvt", [128, CT * C], FP8, isOutput=False)
    wpt = nc.declare_dram_parameter("wpt", [128, CT * C], BF16, isOutput=False)
    w1t = nc.declare_dram_parameter("w1t", [128, HT * CT * 128], BF16, isOutput=False)
    w2t = nc.declare_dram_parameter("w2t", [128, HT * C], BF16, isOutput=False)
    idn = nc.declare_dram_parameter("idn", [128, 128], BF16, isOutput=False)
    out = nc.declare_dram_parameter("out", [QS, C], F32, isOutput=True)

    with tile.TileContext(nc) as tc:
        _build_body(nc, tc, xnt, xm, mm, wqt, wkt, wvt, wpt, w1t, w2t, idn, out)
    _split_excess_waits(nc)
    return nc


def _transpose_128x768(nc, pst_pool, ident, src_bf16, dst, dst_tslice):
    """PE-transpose a (128, 768) bf16 tile into dst[:, 0:CT, dst_tslice]."""
    pst = pst_pool.tile([128, C], BF16, tag="pst", name="pst")
    for dt in range(CT):
        nc.tensor.transpose(
            pst[:, dt * 128:(dt + 1) * 128],
            src_bf16[:, dt * 128:(dt + 1) * 128],
            ident[:],
        )
    nc.scalar.copy(
        out=dst[:, :, dst_tslice],
        in_=pst.rearrange("p (dt q) -> p dt q", dt=CT),
    )


def _build_body(nc, tc, xnt, xm, mm, wqt, wkt, wvt, wpt, w1t, w2t, idn, out):
    from contextlib import ExitStack

    ctx = ExitStack()
    with ctx:
        # ---------------- pools that live to the end of the kernel
        const_p = ctx.enter_context(tc.tile_pool(name="const", bufs=1))
        xmt_p = ctx.enter_context(tc.tile_pool(name="xmtp", bufs=1))
        stats_p = ctx.enter_context(tc.tile_pool(name="statsp", bufs=4))
        ps_p = ctx.enter_context(tc.tile_pool(name="psp", bufs=2, space="PSUM"))

        eps_t = const_p.tile([128, 1], F32, name="eps_t")
        nc.vector.memset(eps_t[:], EPS)
        ident = const_p.tile([128, 128], BF16, name="ident")
        nc.sync.dma_start(out=ident[:], in_=idn[:])
        pools = {"stats": stats_p, "eps": eps_t, "ident": ident}

        xmt = [xmt_p.tile([128, C], F32, tag=f"xmt{i}", name=f"xmt{i}")
               for i in range(QT)]

        # ---------------- pools that live through attention + proj
        oT_p = ctx.enter_context(tc.tile_pool(name="oTp", bufs=1))
        wp_p = ctx.enter_context(tc.tile_pool(name="wpp", bufs=1))
        oTu = oT_p.tile([128, CT, QS], BF16, name="oTu")   # unnormalized
        oT = oT_p.tile([128, CT, QS], BF16, name="oT")     # normalized
        wp_sb = wp_p.tile([128, CT, C], BF16, name="wp_sb")
        cps_ctx = ctx.enter_context(ExitStack())
        pso_p = cps_ctx.enter_context(
            tc.tile_pool(name="psop", bufs=2, space="PSUM"))
        ps2_p = cps_ctx.enter_context(
            tc.tile_pool(name="ps2p", bufs=2, space="PSUM"))

        # ---------------- pools for K/V/Q + attention (released after C)
        kvq_ctx = ctx.enter_context(ExitStack())
        kT_p = kvq_ctx.enter_context(tc.tile_pool(name="kTp", bufs=1))
        v_p = kvq_ctx.enter_context(tc.tile_pool(name="vp", bufs=1))
        qT_p = kvq_ctx.enter_context(tc.tile_pool(name="qTp", bufs=1))
        mm_p = kvq_ctx.enter_context(tc.tile_pool(name="mmp", bufs=1))
        pc_p = kvq_ctx.enter_context(tc.tile_pool(name="pcp", bufs=13))
        z_p = kvq_ctx.enter_context(tc.tile_pool(name="zp", bufs=2))
        zb_p = kvq_ctx.enter_context(tc.tile_pool(name="zbp", bufs=2))
        zd_p = kvq_ctx.enter_context(
            tc.tile_pool(name="zdp", bufs=1, space="DRAM"))
        # xnT freed once the V projection has consumed it (mid-attention);
        # created last so it can be popped first (pools release LIFO).
        xnT_ctx = kvq_ctx.enter_context(ExitStack())
        xnT_p = xnT_ctx.enter_context(tc.tile_pool(name="xnTp", bufs=1))
        wqk_p = xnT_ctx.enter_context(tc.tile_pool(name="wqkp", bufs=1))

        VP = 65   # vaug inner stride: D values + the ones column
        xnT = xnT_p.tile([128, 4, CT, QS], FP8, name="xnT")
        kT = kT_p.tile([128, CT, N], BF16, name="kT")
        vaug = v_p.tile([128, NT, H, VP], BF16, name="vaug")
        qT = qT_p.tile([128, CT, QS], BF16, name="qT")
        mmsb = mm_p.tile([128, NT, QS], BF16, name="mmsb")

        nc.vector.memset(vaug[:, :, :, D:D + 1], 1.0)

        # ---------------- input + weight DMAs (contiguous, multi-queue)
        xntr = xnt.rearrange("p (c kc t) -> p c kc t", c=4, kc=CT)
        for tch in range(4):
            nc.sync.dma_start(out=xnT[:, tch], in_=xntr[:, tch])
        # wk/wq are uploaded per-head-pair (dt-major) so head pair 0's
        # projection can start after a small transfer.
        wkr = wkt.rearrange("p (dt kc q) -> p dt kc q", dt=CT, kc=CT)
        wqr = wqt.rearrange("p (dt kc q) -> p dt kc q", dt=CT, kc=CT)
        wk_sb = wqk_p.tile([128, CT, CT, 128], FP8, tag="wk", name="wk_sb")
        wq_sb = wqk_p.tile([128, CT, CT, 128], FP8, tag="wq", name="wq_sb")
        for dt in range(CT):
            nc.gpsimd.dma_start(out=wk_sb[:, dt], in_=wkr[:, dt])
            nc.gpsimd.dma_start(out=wq_sb[:, dt], in_=wqr[:, dt])
        nc.scalar.dma_start(
            out=mmsb[:], in_=mm.rearrange("p (kc q) -> p kc q", kc=NT)
        )
        wv_sb = wqk_p.tile([128, CT, C], FP8, tag="wv", name="wv_sb")
        nc.gpsimd.dma_start(
            out=wv_sb[:], in_=wvt.rearrange("p (kc d) -> p kc d", kc=CT)
        )
        for t in range(QT):
            nc.sync.dma_start(out=xmt[t][:], in_=xm[t * 128:(t + 1) * 128, :])
        nc.scalar.dma_start(
            out=wp_sb[:], in_=wpt.rearrange("p (kc d) -> p kc d", kc=CT)
        )

        # ---------------- filler units: each emits ~1-2us of independent PE
        # work (plus its PSUM->SBUF evacuation) to keep the PE array dense
        # (and therefore at full clock) inside ACT-bound attention phases.
        def k_unit(dt, tch):
            ps = ps_p.tile([128, 512], F32, tag="ps", name="ps")
            for k2 in range(CT // 2):
                nc.tensor.matmul(
                    ps[:],
                    wk_sb[:, dt, 2 * k2:2 * k2 + 2, :],
                    xnT[:, tch, 2 * k2:2 * k2 + 2, :],
                    start=(k2 == 0), stop=(k2 == CT // 2 - 1),
                    skip_group_check=True, perf_mode=DR,
                )
            nc.vector.tensor_scalar(
                out=kT[:, dt, tch * 512:(tch + 1) * 512], in0=ps[:],
                scalar1=float(PSCALE), scalar2=None, op0=ALU.mult,
            )

        def q_unit(dt):
            ps = ps_p.tile([128, 512], F32, tag="ps", name="ps")
            for k2 in range(CT // 2):
                nc.tensor.matmul(
                    ps[:],
                    wq_sb[:, dt, 2 * k2:2 * k2 + 2, :],
                    xnT[:, 0, 2 * k2:2 * k2 + 2, :],
                    start=(k2 == 0), stop=(k2 == CT // 2 - 1),
                    skip_group_check=True, perf_mode=DR,
                )
            nc.vector.tensor_scalar(
                out=qT[:, dt, :], in0=ps[:],
                scalar1=float(PSCALE), scalar2=None, op0=ALU.mult,
            )

        def v_unit(tt, nch):
            tch, sub = divmod(tt, 4)
            ps = ps_p.tile([128, 384], F32, tag="ps", name="psv")
            for k2 in range(CT // 2):
                nc.tensor.matmul(
                    ps[:],
                    xnT[:, tch, 2 * k2:2 * k2 + 2, sub * 128:(sub + 1) * 128],
                    wv_sb[:, 2 * k2:2 * k2 + 2, nch * 384:(nch + 1) * 384],
                    start=(k2 == 0), stop=(k2 == CT // 2 - 1),
                    skip_group_check=True, perf_mode=DR,
                )
            # v stays at x32 scale inside fp8 vaug (better fp8 resolution);
            # Wp carries the 1/32 on the host.
            nc.vector.tensor_copy(
                out=vaug[:, tt, nch * 6:(nch + 1) * 6, 0:D],
                in_=ps.rearrange("p (h d) -> p h d", h=6),
            )

        # per-head-pair filler rations: during hp i, emit exactly head pair
        # i+1's five K/Q units (one every ~3 steps); hp 0 additionally
        # carries the whole V projection (2 units per step).
        fillers = {}
        for dt in range(1, CT):
            units = [lambda dt=dt, tch=tch: k_unit(dt, tch) for tch in range(4)]
            units.append(lambda dt=dt: q_unit(dt))
            fillers[dt - 1] = units
        v_work = [(tt, nch) for tt in range(NT) for nch in range(2)]

        # head pair 0's K/Q run up front (attention depends on them)
        for tch in range(4):
            k_unit(0, tch)
        q_unit(0)

        # Z bookkeeping: Z rows (PSUM row 64 of each AV accumulator) are
        # copied to DRAM as they appear; two batched reciprocals produce
        # 1/Z spread over 128 partitions... no — batched over head rows,
        # overlapped under later head pairs' compute.
        zdA = zd_p.tile([H, QS], F32, name="zdA", tag="zdA")
        zdR = zd_p.tile([H, QS], F32, name="zdR", tag="zdR")

        def z_batch(h0, h1):
            """Emit fine-grained work items that turn Z rows h0..h1-1 (in
            DRAM) into 1/Z rows: one DMA in, reciprocal in 4 column chunks
            (so no single DVE op exceeds ~1us), one DMA out."""
            nrow = h1 - h0
            zsb = z_p.tile([H, QS], F32, tag="zsb", name="zsb")
            zrb = z_p.tile([H, QS], F32, tag="zrb", name="zrb")
            items = [lambda: nc.gpsimd.dma_start(
                out=zsb[0:nrow, :], in_=zdA[h0:h1, :])]
            for cc in range(4):
                items.append(lambda cc=cc: nc.vector.reciprocal(
                    out=zrb[0:nrow, cc * 128:(cc + 1) * 128],
                    in_=zsb[0:nrow, cc * 128:(cc + 1) * 128],
                ))
            items.append(lambda: nc.gpsimd.dma_start(
                out=zdR[h0:h1, :], in_=zrb[0:nrow, :]))
            return items

        def z_apply(hp):
            """Broadcast 1/Z for head pair hp and normalize oTu -> oT.
            The multiply runs on the (otherwise idle) gpsimd engine."""
            zbig = zb_p.tile([128, 512], F32, tag="zbig", name="zbig")
            for half in range(2):
                nc.gpsimd.dma_start(
                    out=zbig[half * 64:(half + 1) * 64, :],
                    in_=zdR[hp * 2 + half:hp * 2 + half + 1, :]
                    .to_broadcast([64, 512]),
                )
            nc.gpsimd.tensor_tensor(
                out=oT[:, hp, :], in0=oTu[:, hp, :], in1=zbig[:, :],
                op=ALU.mult,
            )

        # ---------------- phase C: attention
        AV_LAG = 5
        # Z normalization batches, emitted as small work items one per
        # attention step: head pairs 0..3 resolve under hp 4, hp 4 under
        # hp 5; only hp 5's own Z (plus proj) remains for the tail.
        post_work = {
            4: z_batch(0, 8) + [lambda h=h: z_apply(h) for h in range(4)],
            5: z_batch(8, 10) + [lambda: z_apply(4)],
        }
        for hp in range(CT):
            psos = [
                pso_p.tile([VP, 512], F32, tag="pso", name="pso"),
                pso_p.tile([VP, 512], F32, tag="pso", name="pso"),
            ]

            def emit_av(half, pc, kc2):
                for j in range(2):
                    kc = kc2 * 2 + j
                    nc.tensor.matmul(
                        psos[half][:],
                        vaug[:, kc, hp * 2 + half, :],
                        pc[:, j, :],
                        start=(kc == 0), stop=(kc == NT - 1),
                        skip_group_check=True,
                    )

            pend = {0: [], 1: []}
            steps_done = 0
            for kc2 in range(NT // 2):
                for half in range(2):
                    p0 = half * 64
                    pss = ps2_p.tile([128, 1024], F32, tag="pss", name="pss")
                    for j in range(2):
                        kc = kc2 * 2 + j
                        nc.tensor.matmul(
                            pss[:, j * 512:(j + 1) * 512],
                            kT[p0:p0 + 64, hp, kc * 128:(kc + 1) * 128],
                            qT[p0:p0 + 64, hp, :],
                            start=True, stop=True,
                        )
                    pc = pc_p.tile([128, 2, QS], BF16, tag="pc", name="pc")
                    nc.scalar.activation(
                        out=pc[:],
                        in_=pss.rearrange("p (two q) -> p two q", two=2),
                        func=AF.Exp,
                    )
                    nc.vector.tensor_mul(
                        pc[:], pc[:], mmsb[:, kc2 * 2:kc2 * 2 + 2, :]
                    )
                    pend[half].append((pc, kc2))
                    if len(pend[half]) > AV_LAG:
                        pcq, k2q = pend[half].pop(0)
                        emit_av(half, pcq, k2q)
                    # PE fillers: V projection inside head pair 0 (its AV
                    # chunks consume vaug progressively), K/Q projections
                    # for head pair hp+1 spread across hp's steps.
                    steps_done += 1
                    if hp == 0:
                        for _ in range(2):
                            if v_work:
                                tt, nch = v_work.pop(0)
                                v_unit(tt, nch)
                    ration = fillers.get(hp, [])
                    if steps_done % 3 == 0 and ration:
                        ration.pop(0)()
                    pw = post_work.get(hp, [])
                    if pw:
                        pw.pop(0)()
            for half in range(2):
                for pcq, k2q in pend[half]:
                    emit_av(half, pcq, k2q)
            # head pair epilogue: evacuate o (unnormalized) and Z
            for half in range(2):
                nc.vector.tensor_copy(
                    out=oTu[half * 64:(half + 1) * 64, hp, :],
                    in_=psos[half][0:64, :],
                )
                zs = z_p.tile([65, 512], F32, tag="zs", name="zs")
                nc.vector.tensor_copy(out=zs[64:65, :], in_=psos[half][64:65, :])
                nc.gpsimd.dma_start(
                    out=zdA[hp * 2 + half:hp * 2 + half + 1, :],
                    in_=zs[64:65, :],
                )
            if hp == 0:
                xnT_ctx.close()

        # tail: only head pair 5's Z remains
        for item in z_batch(10, 12):
            item()
        z_apply(5)

        # ---------------- phase D: proj + residual + LN2 -> xn2T
        kvq_ctx.close()
        cps_ctx.close()
        pst_p = ctx.enter_context(tc.tile_pool(name="pstp", bufs=2, space="PSUM"))
        x1_p = ctx.enter_context(tc.tile_pool(name="x1p", bufs=1))
        xn2T_p = ctx.enter_context(tc.tile_pool(name="xn2Tp", bufs=1))
        w2_p = ctx.enter_context(tc.tile_pool(name="w2p", bufs=1))
        x1t = [x1_p.tile([128, C], F32, tag=f"x1t{i}", name=f"x1t{i}")
               for i in range(QT)]
        xn2T = xn2T_p.tile([128, CT, QS], BF16, name="xn2T")
        w2_sb = w2_p.tile([128, HT, C], BF16, name="w2_sb")
        w2r = w2t.rearrange("p (ht c) -> p ht c", ht=HT)
        for h in range(3):
            nc.gpsimd.dma_start(
                out=w2_sb[:, h * 8:(h + 1) * 8, :], in_=w2r[:, h * 8:(h + 1) * 8, :]
            )
        with tc.tile_pool(name="xn2", bufs=2) as xn2_p:
            for tt in range(QT):
                for nch in range(2):
                    ps = ps_p.tile([128, 384], F32, tag="ps", name="ps")
                    for kc in range(CT):
                        nc.tensor.matmul(
                            ps[:],
                            oT[:, kc, tt * 128:(tt + 1) * 128],
                            wp_sb[:, kc, nch * 384:(nch + 1) * 384],
                            start=(kc == 0), stop=(kc == CT - 1),
                        )
                    nc.vector.scalar_tensor_tensor(
                        out=x1t[tt][:, nch * 384:(nch + 1) * 384],
                        in0=ps[:], scalar=1.0,
                        in1=xmt[tt][:, nch * 384:(nch + 1) * 384],
                        op0=ALU.mult, op1=ALU.add,
                    )
                mu, rstd = _layer_norm_tile(nc, pools, x1t[tt])
                xn2 = xn2_p.tile([128, C], BF16, tag="xn2", name="xn2")
                nc.vector.tensor_scalar(
                    out=xn2[:], in0=x1t[tt][:], scalar1=mu, scalar2=rstd,
                    op0=ALU.subtract, op1=ALU.mult,
                )
                _transpose_128x768(
                    nc, pst_p, ident, xn2, xn2T, slice(tt * 128, (tt + 1) * 128)
                )

        # ---------------- phase E: MLP. fc2 accumulation for the first two
        # token tiles rides along inside the fc1 loop so the PE never waits
        # for the full gelu sweep.
        with tc.tile_pool(name="gTp", bufs=1) as gT_p, \
             tc.tile_pool(name="w1p", bufs=4) as w1_p, \
             tc.tile_pool(name="psE", bufs=4, space="PSUM") as psE_p, \
             tc.tile_pool(name="op", bufs=2) as o_p:
            gT = gT_p.tile([128, HT, QS], BF16, name="gT")
            w1r = w1t.rearrange("p (ht kc q) -> p ht kc q", ht=HT, kc=CT)
            NEARLY = 2
            chains = {}
            for tt in range(NEARLY):
                for nch in range(2):
                    chains[(tt, nch)] = psE_p.tile(
                        [128, 384], F32, tag="psE", name="psE"
                    )
            for ht in range(HT):
                w1c = w1_p.tile([128, CT, 128], BF16, tag="w1c", name="w1c")
                nc.sync.dma_start(out=w1c[:], in_=w1r[:, ht])
                ps = ps_p.tile([128, 512], F32, tag="ps", name="ps")
                for kc in range(CT):
                    nc.tensor.matmul(
                        ps[:],
                        w1c[:, kc, :],
                        xn2T[:, kc, :],
                        start=(kc == 0), stop=(kc == CT - 1),
                    )
                nc.scalar.activation(out=gT[:, ht, :], in_=ps[:], func=AF.Gelu)
                for tt in range(NEARLY):
                    for nch in range(2):
                        nc.tensor.matmul(
                            chains[(tt, nch)][:],
                            gT[:, ht, tt * 128:(tt + 1) * 128],
                            w2_sb[:, ht, nch * 384:(nch + 1) * 384],
                            start=(ht == 0), stop=(ht == HT - 1),
                            skip_group_check=True,
                        )
            for tt in range(QT):
                outt = o_p.tile([128, C], F32, tag="outt", name="outt")
                for nch in range(2):
                    if tt < NEARLY:
                        ps2 = chains[(tt, nch)]
                    else:
                        ps2 = psE_p.tile([128, 384], F32, tag="psE", name="psE")
                        for ht in range(HT):
                            nc.tensor.matmul(
                                ps2[:],
                                gT[:, ht, tt * 128:(tt + 1) * 128],
                                w2_sb[:, ht, nch * 384:(nch + 1) * 384],
                                start=(ht == 0), stop=(ht == HT - 1),
                            )
                    nc.vector.scalar_tensor_tensor(
                        out=outt[:, nch * 384:(nch + 1) * 384],
                        in0=ps2[:], scalar=1.0,
                        in1=x1t[tt][:, nch * 384:(nch + 1) * 384],
                        op0=ALU.mult, op1=ALU.add,
                    )
                # spread output DMAs across queues so the final transfers
                # overlap instead of serializing on one queue
                eng = [nc.sync, nc.gpsimd, nc.scalar, nc.sync][tt]
                eng.dma_start(
                    out=out[tt * 128:(tt + 1) * 128, :], in_=outt[:]
                )


# ---------------------------------------------------------------- host side
_CACHED_NC = None


def _get_nc():
    global _CACHED_NC
    if _CACHED_NC is None:
        _CACHED_NC = build_program()
    return _CACHED_NC


def _part_major(a, inner_shape):
    """(CT*128, X) row-major -> (128, prod(inner_shape)) where the leading
    dim is split (blk, 128) and partitions become major: out[p, blk, :] =
    a[blk*128 + p, :]."""
    nblk = a.shape[0] // 128
    return np.ascontiguousarray(
        a.reshape((nblk, 128) + a.shape[1:]).swapaxes(0, 1).reshape(128, -1)
    )


def make_in_maps(x, mask, g1, b1, Wq, Wkv, Wp, bp, g2, b2, W1, bf1, W2, bf2):
    f32 = np.float32
    bf = ml_dtypes.bfloat16
    x = np.asarray(x, f32)
    mask = np.asarray(mask, f32)
    g1 = np.asarray(g1, f32); b1 = np.asarray(b1, f32)
    g2 = np.asarray(g2, f32); b2 = np.asarray(b2, f32)
    Wq = np.asarray(Wq, f32); Wkv = np.asarray(Wkv, f32); Wp = np.asarray(Wp, f32)
    W1 = np.asarray(W1, f32); W2 = np.asarray(W2, f32)
    bp = np.asarray(bp, f32); bf1 = np.asarray(bf1, f32); bf2 = np.asarray(bf2, f32)

    Wk, Wv = Wkv[:C], Wkv[C:]
    # fold LN gains + attention scale into the weights; biases must be zero
    # (they are, for this problem's setup_inputs) for this fast path.
    zero_rows = [
        (b1 @ Wq.T) * SCALE, b1 @ Wk.T, b1 @ Wv.T, bp,
        bf1 + b2 @ W1.T, bf2,
    ]
    for r in zero_rows:
        assert np.abs(r).max() == 0.0, "nonzero bias path not implemented"

    # device layouts ------------------------------------------------------
    f8 = ml_dtypes.float8_e4m3fn
    # wk/wq: [128, dt, kc, 128] with wk[p, dt, kc, q] = WkT[kc*128+p, dt*128+q]
    # QKV weights are uploaded fp8 at x8 scale (see XSCALE/WSCALE notes).
    wkT = (Wk * g1[None, :]).T * WSCALE   # (C in-feat, C out-feat)
    wqT = (Wq * g1[None, :] * SCALE).T * WSCALE
    wk_h = _part_major(wkT, None).reshape(128, CT, CT, 128)   # p, kc, dt, q
    wq_h = _part_major(wqT, None).reshape(128, CT, CT, 128)
    wk_h = np.ascontiguousarray(wk_h.swapaxes(1, 2)).reshape(128, -1).astype(f8)
    wq_h = np.ascontiguousarray(wq_h.swapaxes(1, 2)).reshape(128, -1).astype(f8)
    wv_h = _part_major((Wv * g1[None, :]).T * WSCALE, None).astype(f8)
    # v arrives at x(XSCALE*WSCALE) scale; fold the inverse into Wp
    wp_h = _part_major(Wp.T * PSCALE, None).astype(bf)
    # w1: [128, ht, kc, 128]: w1[p, ht, kc, q] = W1T[kc*128+p, ht*128+q]
    w1T = (W1 * g2[None, :]).T            # (C, HID)
    w1_h = _part_major(w1T, None).reshape(128, CT, HT, 128)
    w1_h = np.ascontiguousarray(w1_h.swapaxes(1, 2)).reshape(128, -1).astype(bf)
    w2_h = _part_major(W2.T, None).astype(bf)                  # p,(ht c)
    idn_h = np.eye(128, dtype=bf)

    # host-side LN1 (plain: gains/biases are folded into the weights above)
    mu = x.mean(axis=-1, keepdims=True)
    var = x.var(axis=-1, keepdims=True)
    xn_full = (x - mu) / np.sqrt(var + EPS)

    in_maps = []
    for c in range(NCORES):
        b, qi = divmod(c, 4)
        q0 = qi * QS
        xr = np.roll(x[b], -q0, axis=0)                    # my tokens first
        xnr = np.roll(xn_full[b], -q0, axis=0)
        # xnt: chunk-major feature-major: [128, tch, kc, 512]
        # xnt[p, tch, kc, t] = XSCALE * xn[tch*512 + t, kc*128 + p]
        xnt_h = np.ascontiguousarray(
            (xnr.T * XSCALE)
            .reshape(CT, 128, 4, QS).transpose(1, 2, 0, 3).reshape(128, -1)
        ).astype(f8)
        km = np.roll(1.0 - mask[b].T, -q0, axis=0)         # keys rolled too
        mmc = _part_major(
            np.ascontiguousarray(km[:, q0:q0 + QS]), None
        ).astype(bf)
        in_maps.append({
            "xnt": xnt_h,
            "xm": np.ascontiguousarray(xr[:QS]),
            "mm": mmc,
            "wqt": wq_h, "wkt": wk_h, "wvt": wv_h, "wpt": wp_h,
            "w1t": w1_h, "w2t": w2_h, "idn": idn_h,
        })
    return in_maps


def kernel(**inputs):
    nc = _get_nc()
    in_maps = make_in_maps(**inputs)
    res = run_bass_kernel_spmd(nc, in_maps, core_ids=list(range(NCORES)))
    out = np.empty((B, N, C), np.float32)
    for c in range(NCORES):
        b, qi = divmod(c, 4)
        q0 = qi * QS
        out[b, q0:q0 + QS] = res.results[c]["out"]
    return out


if __name__ == "__main__":
    print("building program...")
    nc = _get_nc()
    print("instructions:", sum(len(bb.instructions) for bb in nc.main_func.blocks))


# revision 47
# speedup vs baseline: 1.1372x; 1.0627x over previous
"""Trainium2 Bass kernel for a dense pre-norm transformer block.

Problem: B=2, N=2048, C=768, H=12 heads (D=64), MLP hidden 3072, f32 I/O.

Sharding (8 cores, no collectives): query-parallel. Core c handles batch
c//4 and query rows (c%4)*512 .. +512, for all heads. Each core computes
K/V for its full batch redundantly (4x) — cheaper than cross-core
collectives at these sizes. Each core's x is uploaded rolled so that its
own 512 query tokens are rows 0..511 (attention is permutation-invariant
over keys once the mask is rolled the same way), which keeps the device
program identical across cores.

v4 design notes:
- LN1 runs on the host; xn is uploaded directly in the feature-major
  (transposed) layout the projections consume, so the device starts
  matmulling ~immediately.
- Every DMA source is laid out on the host to be contiguous per
  partition; strided patterns made the descriptor generation (software
  dynamic DMA) take microseconds of engine time per transfer.
- The K/Q projections for head pair i+1 and the whole V projection are
  interleaved into earlier head pairs' attention streams as PE filler:
  attention alone leaves the PE at ~55% duty, which lets the HAM clock
  gate re-throttle the PE to 1.2 GHz and double every matmul.
- Softmax Z rows are collected in DRAM; two batched reciprocals (one
  overlapped under head pair 4-5, one in the tail) replace per-row
  reciprocals that would serialize the vector engine for 3.3us each.

Precision: bf16 matmul operands, f32 PSUM accumulation, f32 layernorm
stats and residuals. LN gains (g1/g2) and the attention 1/sqrt(D) scale
are folded into the weight matrices on the host. All LN/projection biases
in this problem are exactly zero (verified on host at call time).
"""

import os
import sys

for _p in ("/opt/trn_rl_repo",):
    if os.path.isdir(_p) and _p not in sys.path:
        sys.path.append(_p)

import numpy as np
import ml_dtypes

import concourse.bass as bass
import concourse.mybir as mybir
import concourse.tile as tile
from concourse.bass_utils import run_bass_kernel_spmd

# ---------------------------------------------------------------- constants
B, N, C = 2, 2048, 768
H, D = 12, 64
HID = 4 * C
SCALE = D ** -0.5
EPS = 1e-5
NCORES = 8
QS = N // 4          # queries per core = 512
QT = QS // 128       # query token tiles per core = 4
NT = N // 128        # token tiles per batch = 16
CT = C // 128        # feature tiles = 6
HT = HID // 128      # hidden tiles = 24

F32 = mybir.dt.float32
BF16 = mybir.dt.bfloat16
FP8 = mybir.dt.float8e4
DR = mybir.MatmulPerfMode.DoubleRow
AF = mybir.ActivationFunctionType
ALU = mybir.AluOpType

# fp8 scale folding: xn is uploaded x4 and the QKV weights x8 so almost no
# value lands in the e4m3 subnormal range; the x32 product scale is divided
# back out at the kT/qT evacuation and (for the V path) folded into Wp.
XSCALE = 4.0
WSCALE = 8.0
PSCALE = 1.0 / (XSCALE * WSCALE)
EXP_BIAS = -3.0   # keeps exp() outputs inside fp8 range; cancels via 1/Z


def _patch_tile_drain():
    """This walrus build rejects Drain instructions carrying >1 sem-wait
    ("Too many sync wait commands"). Split the TileContext exit-drain's
    waits across a chain of single-wait drains."""
    import concourse.tile as tile_mod

    if getattr(tile_mod.TileContext, "_ant_drain_patched", False):
        return

    def _patched(self, tick_clock, wait_clock):
        nc = self.nc
        drain_inst = nc.sync.drain()
        wait_clock.add_sem_waits(
            drain_inst.ins, tile_mod.ScopedClock({None: tick_clock.global_clock})
        )
        si = drain_inst.ins.sync_info
        if si is not None and si.on_wait and len(si.on_wait) > 1:
            extra = list(si.on_wait[1:])
            si.on_wait = [si.on_wait[0]]
            for w in extra:
                d2 = nc.sync.drain().ins
                si2 = d2.sync_info
                if si2 is None:
                    d2.sync_info = type(si)(on_wait=[w], on_update=[])
                else:
                    si2.on_wait = [w]
        nc.all_engine_barrier()
        assert self.sems is not None
        popped = nc._tile_sem_poison_stack.pop()
        assert popped is self._sem_poison
        nc.clear_and_free_semaphores(list(self.sems.allocated().values()))
        nc.all_engine_barrier()

    tile_mod.TileContext._drain_and_barrier = _patched
    tile_mod.TileContext._ant_drain_patched = True


_MAX_WAITS_BY_TYPE = {"InstDrain": 1, "InstDmaTransposeAnt": 1}
_DEFAULT_MAX_WAITS = 1


def _split_excess_waits(nc):
    """This walrus build rejects instructions carrying more than ~1 sem-wait
    ("Too many sync wait commands"). Move excess waits onto same-engine NOPs
    inserted immediately before the instruction."""
    nid = [0]

    def mk_nop(engine, wait):
        nid[0] += 1
        nop = mybir.InstNoOp(name=f"antw-{nid[0]}", ins=[], outs=[])
        nop.engine = engine
        nop.sync_info = mybir.SyncInfo(on_wait=[wait], on_update=[])
        return nop

    for bb in nc.main_func.blocks:
        new_list = []
        for ins in bb.instructions:
            si = ins.sync_info
            lim = _MAX_WAITS_BY_TYPE.get(type(ins).__name__, _DEFAULT_MAX_WAITS)
            if si is not None and si.on_wait and len(si.on_wait) > lim:
                extra = list(si.on_wait[lim:])
                si.on_wait = list(si.on_wait[:lim])
                for w in extra:
                    new_list.append(mk_nop(ins.engine, w))
            new_list.append(ins)
        bb.instructions[:] = new_list


def _layer_norm_tile(nc, pools, xt, rows=128):
    """LN stats for one (128, C) f32 tile -> (mu, rstd) per-partition aps."""
    spool = pools["stats"]
    stats = spool.tile([128, 3, 6], F32, tag="stats", name="stats")
    for sg in range(3):
        nc.vector.bn_stats(
            out=stats[:rows, sg, :], in_=xt[:rows, sg * 256:(sg + 1) * 256]
        )
    mv = spool.tile([128, 2], F32, tag="mv", name="mv")
    nc.vector.bn_aggr(out=mv[:rows], in_=stats[:rows])
    rstd = spool.tile([128, 1], F32, tag="rstd", name="rstd")
    nc.scalar.activation(
        out=rstd[:rows], in_=mv[:rows, 1:2], func=AF.Sqrt, bias=pools["eps"][:rows]
    )
    rstd2 = spool.tile([128, 1], F32, tag="rstd2", name="rstd2")
    nc.vector.reciprocal(out=rstd2[:rows], in_=rstd[:rows])
    return mv[:rows, 0:1], rstd2[:rows]


def build_program():
    """Build the SPMD single-core program (same BIR for all 8 cores)."""
    _patch_tile_drain()
    nc = bass.Bass()

    # Host-side layouts are exactly the SBUF layouts (contiguous per
    # partition) so every transfer is a fast hardware-dynamic DMA.
    xnt = nc.declare_dram_parameter("xnt", [128, 4 * CT * QS], FP8, isOutput=False)
    xm = nc.declare_dram_parameter("xm", [QS, C], F32, isOutput=False)
    mm = nc.declare_dram_parameter("mm", [128, NT * QS], BF16, isOutput=False)
    wqt = nc.declare_dram_parameter("wqt", [128, CT * CT * 128], FP8, isOutput=False)
    wkt = nc.declare_dram_parameter("wkt", [128, CT * CT * 128], FP8, isOutput=False)
    wvt = nc.declare_dram_parameter("wvt", [128, CT * C], FP8, isOutput=False)
    wpt = nc.declare_dram_parameter("wpt", [128, CT * C], BF16, isOutput=False)
    w1t = nc.declare_dram_parameter("w1t", [128, HT * CT * 128], BF16, isOutput=False)
    w2t = nc.declare_dram_parameter("w2t", [128, HT * C], BF16, isOutput=False)
    idn = nc.declare_dram_parameter("idn", [128, 128], BF16, isOutput=False)
    out = nc.declare_dram_parameter("out", [QS, C], F32, isOutput=True)

    with tile.TileContext(nc) as tc:
        _build_body(nc, tc, xnt, xm, mm, wqt, wkt, wvt, wpt, w1t, w2t, idn, out)
    _split_excess_waits(nc)
    return nc


def _transpose_128x768(nc, pst_pool, ident, src_bf16, dst, dst_tslice):
    """PE-transpose a (128, 768) bf16 tile into dst[:, 0:CT, dst_tslice]."""
    pst = pst_pool.tile([128, C], BF16, tag="pst", name="pst")
    for dt in range(CT):
        nc.tensor.transpose(
            pst[:, dt * 128:(dt + 1) * 128],
            src_bf16[:, dt * 128:(dt + 1) * 128],
            ident[:],
        )
    nc.scalar.copy(
        out=dst[:, :, dst_tslice],
        in_=pst.rearrange("p (dt q) -> p dt q", dt=CT),
    )


def _build_body(nc, tc, xnt, xm, mm, wqt, wkt, wvt, wpt, w1t, w2t, idn, out):
    from contextlib import ExitStack

    ctx = ExitStack()
    with ctx:
        # ---------------- pools that live to the end of the kernel
        const_p = ctx.enter_context(tc.tile_pool(name="const", bufs=1))
        xmt_p = ctx.enter_context(tc.tile_pool(name="xmtp", bufs=1))
        stats_p = ctx.enter_context(tc.tile_pool(name="statsp", bufs=4))
        ps_p = ctx.enter_context(tc.tile_pool(name="psp", bufs=2, space="PSUM"))

        eps_t = const_p.tile([128, 1], F32, name="eps_t")
        nc.vector.memset(eps_t[:], EPS)
        pscale_t = const_p.tile([128, 1], F32, name="pscale_t")
        nc.vector.memset(pscale_t[:], PSCALE)
        ident = const_p.tile([128, 128], BF16, name="ident")
        nc.sync.dma_start(out=ident[:], in_=idn[:])
        pools = {"stats": stats_p, "eps": eps_t, "ident": ident}

        xmt = [xmt_p.tile([128, C], F32, tag=f"xmt{i}", name=f"xmt{i}")
               for i in range(QT)]

        # ---------------- pools that live through attention + proj
        oT_p = ctx.enter_context(tc.tile_pool(name="oTp", bufs=1))
        wp_p = ctx.enter_context(tc.tile_pool(name="wpp", bufs=1))
        oTu = oT_p.tile([128, CT, QS], BF16, name="oTu")   # unnormalized
        oT = oT_p.tile([128, CT, QS], BF16, name="oT")     # normalized
        wp_sb = wp_p.tile([128, CT, C], BF16, name="wp_sb")
        cps_ctx = ctx.enter_context(ExitStack())
        pso_p = cps_ctx.enter_context(
            tc.tile_pool(name="psop", bufs=2, space="PSUM"))
        ps2_p = cps_ctx.enter_context(
            tc.tile_pool(name="ps2p", bufs=2, space="PSUM"))

        # ---------------- pools for K/V/Q + attention (released after C)
        kvq_ctx = ctx.enter_context(ExitStack())
        kT_p = kvq_ctx.enter_context(tc.tile_pool(name="kTp", bufs=1))
        v_p = kvq_ctx.enter_context(tc.tile_pool(name="vp", bufs=1))
        qT_p = kvq_ctx.enter_context(tc.tile_pool(name="qTp", bufs=1))
        mm_p = kvq_ctx.enter_context(tc.tile_pool(name="mmp", bufs=1))
        pc_p = kvq_ctx.enter_context(tc.tile_pool(name="pcp", bufs=13))
        z_p = kvq_ctx.enter_context(tc.tile_pool(name="zp", bufs=2))
        zb_p = kvq_ctx.enter_context(tc.tile_pool(name="zbp", bufs=2))
        zd_p = kvq_ctx.enter_context(
            tc.tile_pool(name="zdp", bufs=1, space="DRAM"))
        # xnT freed once the V projection has consumed it (mid-attention);
        # created last so it can be popped first (pools release LIFO).
        xnT_ctx = kvq_ctx.enter_context(ExitStack())
        xnT_p = xnT_ctx.enter_context(tc.tile_pool(name="xnTp", bufs=1))
        wqk_p = xnT_ctx.enter_context(tc.tile_pool(name="wqkp", bufs=1))

        VP = 65   # vaug inner stride: D values + the ones column
        xnT = xnT_p.tile([128, 4, CT, QS], FP8, name="xnT")
        kT = kT_p.tile([128, CT, N], BF16, name="kT")
        vaug = v_p.tile([128, NT, H, VP], BF16, name="vaug")
        qT = qT_p.tile([128, CT, QS], BF16, name="qT")
        mmsb = mm_p.tile([128, NT, QS], BF16, name="mmsb")

        nc.vector.memset(vaug[:, :, :, D:D + 1], 1.0)

        # ---------------- input + weight DMAs (contiguous, multi-queue)
        xntr = xnt.rearrange("p (c kc t) -> p c kc t", c=4, kc=CT)
        for tch in range(4):
            nc.gpsimd.dma_start(out=xnT[:, tch], in_=xntr[:, tch])
        # wk/wq are uploaded per-head-pair (dt-major) so head pair 0's
        # projection can start after a small transfer.
        wkr = wkt.rearrange("p (dt kc q) -> p dt kc q", dt=CT, kc=CT)
        wqr = wqt.rearrange("p (dt kc q) -> p dt kc q", dt=CT, kc=CT)
        wk_sb = wqk_p.tile([128, CT, CT, 128], FP8, tag="wk", name="wk_sb")
        wq_sb = wqk_p.tile([128, CT, CT, 128], FP8, tag="wq", name="wq_sb")
        for dt in range(CT):
            nc.sync.dma_start(out=wk_sb[:, dt], in_=wkr[:, dt])
            nc.sync.dma_start(out=wq_sb[:, dt], in_=wqr[:, dt])
        nc.gpsimd.dma_start(
            out=mmsb[:], in_=mm.rearrange("p (kc q) -> p kc q", kc=NT)
        )
        wv_sb = wqk_p.tile([128, CT, C], FP8, tag="wv", name="wv_sb")
        nc.scalar.dma_start(
            out=wv_sb[:], in_=wvt.rearrange("p (kc d) -> p kc d", kc=CT)
        )
        for t in range(QT):
            nc.scalar.dma_start(out=xmt[t][:], in_=xm[t * 128:(t + 1) * 128, :])
        nc.scalar.dma_start(
            out=wp_sb[:], in_=wpt.rearrange("p (kc d) -> p kc d", kc=CT)
        )

        # ---------------- filler units: each emits ~1-2us of independent PE
        # work (plus its PSUM->SBUF evacuation) to keep the PE array dense
        # (and therefore at full clock) inside ACT-bound attention phases.
        def k_unit(dt, tch):
            ps = ps_p.tile([128, 512], F32, tag="ps", name="ps")
            for k2 in range(CT // 2):
                nc.tensor.matmul(
                    ps[:],
                    wk_sb[:, dt, 2 * k2:2 * k2 + 2, :],
                    xnT[:, tch, 2 * k2:2 * k2 + 2, :],
                    start=(k2 == 0), stop=(k2 == CT // 2 - 1),
                    skip_group_check=True, perf_mode=DR,
                )
            nc.vector.tensor_scalar(
                out=kT[:, dt, tch * 512:(tch + 1) * 512], in0=ps[:],
                scalar1=float(PSCALE), scalar2=None, op0=ALU.mult,
            )

        def q_unit(dt):
            ps = ps_p.tile([128, 512], F32, tag="ps", name="ps")
            for k2 in range(CT // 2):
                nc.tensor.matmul(
                    ps[:],
                    wq_sb[:, dt, 2 * k2:2 * k2 + 2, :],
                    xnT[:, 0, 2 * k2:2 * k2 + 2, :],
                    start=(k2 == 0), stop=(k2 == CT // 2 - 1),
                    skip_group_check=True, perf_mode=DR,
                )
            nc.vector.tensor_scalar(
                out=qT[:, dt, :], in0=ps[:],
                scalar1=float(PSCALE), scalar2=None, op0=ALU.mult,
            )

        def v_unit(tt, nch):
            tch, sub = divmod(tt, 4)
            ps = ps_p.tile([128, 384], F32, tag="ps", name="psv")
            for k2 in range(CT // 2):
                nc.tensor.matmul(
                    ps[:],
                    xnT[:, tch, 2 * k2:2 * k2 + 2, sub * 128:(sub + 1) * 128],
                    wv_sb[:, 2 * k2:2 * k2 + 2, nch * 384:(nch + 1) * 384],
                    start=(k2 == 0), stop=(k2 == CT // 2 - 1),
                    skip_group_check=True, perf_mode=DR,
                )
            # v stays at x32 scale inside fp8 vaug (better fp8 resolution);
            # Wp carries the 1/32 on the host.
            nc.vector.tensor_copy(
                out=vaug[:, tt, nch * 6:(nch + 1) * 6, 0:D],
                in_=ps.rearrange("p (h d) -> p h d", h=6),
            )

        # per-head-pair filler rations: during hp i, emit exactly head pair
        # i+1's five K/Q units (one every ~3 steps); hp 0 additionally
        # carries the whole V projection (2 units per step).
        fillers = {}
        for dt in range(1, CT):
            units = [lambda dt=dt, tch=tch: k_unit(dt, tch) for tch in range(4)]
            units.append(lambda dt=dt: q_unit(dt))
            fillers[dt - 1] = units
        v_work = [(tt, nch) for tt in range(NT) for nch in range(2)]

        # head pair 0's K/Q run up front (attention depends on them)
        for tch in range(4):
            k_unit(0, tch)
        q_unit(0)

        # Z bookkeeping: Z rows (PSUM row 64 of each AV accumulator) are
        # copied to DRAM as they appear; two batched reciprocals produce
        # 1/Z spread over 128 partitions... no — batched over head rows,
        # overlapped under later head pairs' compute.
        zdA = zd_p.tile([H, QS], F32, name="zdA", tag="zdA")
        zdR = zd_p.tile([H, QS], F32, name="zdR", tag="zdR")

        def z_batch(h0, h1):
            """Emit fine-grained work items that turn Z rows h0..h1-1 (in
            DRAM) into 1/Z rows: one DMA in, reciprocal in 4 column chunks
            (so no single DVE op exceeds ~1us), one DMA out."""
            nrow = h1 - h0
            zsb = z_p.tile([H, QS], F32, tag="zsb", name="zsb")
            zrb = z_p.tile([H, QS], F32, tag="zrb", name="zrb")
            items = [lambda: nc.gpsimd.dma_start(
                out=zsb[0:nrow, :], in_=zdA[h0:h1, :])]
            for cc in range(4):
                items.append(lambda cc=cc: nc.vector.reciprocal(
                    out=zrb[0:nrow, cc * 128:(cc + 1) * 128],
                    in_=zsb[0:nrow, cc * 128:(cc + 1) * 128],
                ))
            items.append(lambda: nc.gpsimd.dma_start(
                out=zdR[h0:h1, :], in_=zrb[0:nrow, :]))
            return items

        def z_apply(hp):
            """Broadcast 1/Z for head pair hp and normalize oTu -> oT.
            The multiply runs on the (otherwise idle) gpsimd engine."""
            zbig = zb_p.tile([128, 512], F32, tag="zbig", name="zbig")
            for half in range(2):
                nc.gpsimd.dma_start(
                    out=zbig[half * 64:(half + 1) * 64, :],
                    in_=zdR[hp * 2 + half:hp * 2 + half + 1, :]
                    .to_broadcast([64, 512]),
                )
            nc.gpsimd.tensor_tensor(
                out=oT[:, hp, :], in0=oTu[:, hp, :], in1=zbig[:, :],
                op=ALU.mult,
            )

        # ---------------- phase C: attention
        AV_LAG = 5
        # Z normalization batches, emitted as small work items one per
        # attention step: head pairs 0..3 resolve under hp 4, hp 4 under
        # hp 5; only hp 5's own Z (plus proj) remains for the tail.
        post_work = {
            4: z_batch(0, 8) + [lambda h=h: z_apply(h) for h in range(4)],
            5: z_batch(8, 10) + [lambda: z_apply(4)],
        }
        for hp in range(CT):
            psos = [
                pso_p.tile([VP, 512], F32, tag="pso", name="pso"),
                pso_p.tile([VP, 512], F32, tag="pso", name="pso"),
            ]

            def emit_av(half, pc, kc2):
                for j in range(2):
                    kc = kc2 * 2 + j
                    nc.tensor.matmul(
                        psos[half][:],
                        vaug[:, kc, hp * 2 + half, :],
                        pc[:, j, :],
                        start=(kc == 0), stop=(kc == NT - 1),
                        skip_group_check=True,
                    )

            pend = {0: [], 1: []}
            steps_done = 0
            for kc2 in range(NT // 2):
                for half in range(2):
                    p0 = half * 64
                    pss = ps2_p.tile([128, 1024], F32, tag="pss", name="pss")
                    for j in range(2):
                        kc = kc2 * 2 + j
                        nc.tensor.matmul(
                            pss[:, j * 512:(j + 1) * 512],
                            kT[p0:p0 + 64, hp, kc * 128:(kc + 1) * 128],
                            qT[p0:p0 + 64, hp, :],
                            start=True, stop=True,
                        )
                    pc = pc_p.tile([128, 2, QS], BF16, tag="pc", name="pc")
                    nc.scalar.activation(
                        out=pc[:],
                        in_=pss.rearrange("p (two q) -> p two q", two=2),
                        func=AF.Exp,
                    )
                    nc.vector.tensor_mul(
                        pc[:], pc[:], mmsb[:, kc2 * 2:kc2 * 2 + 2, :]
                    )
                    pend[half].append((pc, kc2))
                    if len(pend[half]) > AV_LAG:
                        pcq, k2q = pend[half].pop(0)
                        emit_av(half, pcq, k2q)
                    # PE fillers: V projection inside head pair 0 (its AV
                    # chunks consume vaug progressively), K/Q projections
                    # for head pair hp+1 spread across hp's steps.
                    steps_done += 1
                    if hp == 0:
                        for _ in range(2):
                            if v_work:
                                tt, nch = v_work.pop(0)
                                v_unit(tt, nch)
                    ration = fillers.get(hp, [])
                    if steps_done % 3 == 0 and ration:
                        ration.pop(0)()
                    pw = post_work.get(hp, [])
                    if pw:
                        pw.pop(0)()
            for half in range(2):
                for pcq, k2q in pend[half]:
                    emit_av(half, pcq, k2q)
            # head pair epilogue: evacuate o (unnormalized) and Z
            for half in range(2):
                nc.vector.tensor_copy(
                    out=oTu[half * 64:(half + 1) * 64, hp, :],
                    in_=psos[half][0:64, :],
                )
                zs = z_p.tile([65, 512], F32, tag="zs", name="zs")
                nc.vector.tensor_copy(out=zs[64:65, :], in_=psos[half][64:65, :])
                nc.gpsimd.dma_start(
                    out=zdA[hp * 2 + half:hp * 2 + half + 1, :],
                    in_=zs[64:65, :],
                )
            if hp == 0:
                xnT_ctx.close()

        # tail: only head pair 5's Z remains
        for item in z_batch(10, 12):
            item()
        z_apply(5)

        # ---------------- phase D: proj + residual + LN2 -> xn2T.
        # The first four proj chains run their kc 0..4 partials immediately
        # (only head pair 5's Z is still resolving) so the PE keeps working
        # through the softmax tail.
        kvq_ctx.close()
        cps_ctx.close()
        d_ctx = ExitStack()
        x1_p = ctx.enter_context(tc.tile_pool(name="x1p", bufs=1))
        xn2T_p = ctx.enter_context(tc.tile_pool(name="xn2Tp", bufs=1))
        w2_p = ctx.enter_context(tc.tile_pool(name="w2p", bufs=1))
        x1t = [x1_p.tile([128, C], F32, tag=f"x1t{i}", name=f"x1t{i}")
               for i in range(QT)]
        xn2T = xn2T_p.tile([128, CT, QS], BF16, name="xn2T")
        w2_sb = w2_p.tile([128, HT, C], BF16, name="w2_sb")
        w2r = w2t.rearrange("p (ht c) -> p ht c", ht=HT)
        for h in range(3):
            nc.gpsimd.dma_start(
                out=w2_sb[:, h * 8:(h + 1) * 8, :], in_=w2r[:, h * 8:(h + 1) * 8, :]
            )
        with d_ctx:
            pst_p = d_ctx.enter_context(
                tc.tile_pool(name="pstp", bufs=2, space="PSUM"))
            projp = d_ctx.enter_context(
                tc.tile_pool(name="projp", bufs=4, space="PSUM"))
            xn2_p = d_ctx.enter_context(tc.tile_pool(name="xn2", bufs=2))

            def proj_mm(ch, tt, nch, kc, stop):
                nc.tensor.matmul(
                    ch[:],
                    oT[:, kc, tt * 128:(tt + 1) * 128],
                    wp_sb[:, kc, nch * 384:(nch + 1) * 384],
                    start=(kc == 0), stop=stop, skip_group_check=True,
                )

            def proj_stt(ch, tt, nch):
                nc.vector.scalar_tensor_tensor(
                    out=x1t[tt][:, nch * 384:(nch + 1) * 384],
                    in0=ch[:], scalar=1.0,
                    in1=xmt[tt][:, nch * 384:(nch + 1) * 384],
                    op0=ALU.mult, op1=ALU.add,
                )

            def ln2(tt):
                mu, rstd = _layer_norm_tile(nc, pools, x1t[tt])
                xn2 = xn2_p.tile([128, C], BF16, tag="xn2", name="xn2")
                nc.vector.tensor_scalar(
                    out=xn2[:], in0=x1t[tt][:], scalar1=mu, scalar2=rstd,
                    op0=ALU.subtract, op1=ALU.mult,
                )
                _transpose_128x768(
                    nc, pst_p, ident, xn2, xn2T, slice(tt * 128, (tt + 1) * 128)
                )

            first = [(0, 0), (0, 1), (1, 0), (1, 1)]
            second = [(2, 0), (2, 1), (3, 0), (3, 1)]
            chains = {}
            for tt, nch in first:
                ch = chains[(tt, nch)] = projp.tile(
                    [128, 384], F32, tag="proj", name="proj")
                for kc in range(CT - 1):
                    proj_mm(ch, tt, nch, kc, stop=False)
            for tt, nch in first:
                proj_mm(chains[(tt, nch)], tt, nch, CT - 1, stop=True)
            for tt, nch in first:
                proj_stt(chains[(tt, nch)], tt, nch)
            for tt, nch in second:
                ch = chains[(tt, nch)] = projp.tile(
                    [128, 384], F32, tag="proj", name="proj")
                for kc in range(CT):
                    proj_mm(ch, tt, nch, kc, stop=(kc == CT - 1))
            ln2(0)
            ln2(1)
            for tt, nch in second:
                proj_stt(chains[(tt, nch)], tt, nch)
            ln2(2)
            ln2(3)

        # ---------------- phase E: MLP, fp8 DoubleRow. fc2 accumulation for
        # the first three token tiles rides along inside the fc1 loop so the
        # PE never waits for the full gelu sweep.
        with tc.tile_pool(name="gTp", bufs=1) as gT_p, \
             tc.tile_pool(name="w1p", bufs=4) as w1_p, \
             tc.tile_pool(name="psE", bufs=6, space="PSUM") as psE_p, \
             tc.tile_pool(name="op", bufs=2) as o_p:
            gT = gT_p.tile([128, HT, QS], BF16, name="gT")
            w1r = w1t.rearrange("p (ht kc q) -> p ht kc q", ht=HT, kc=CT)
            NEARLY = 3
            chains = {}
            for tt in range(NEARLY):
                for nch in range(2):
                    chains[(tt, nch)] = psE_p.tile(
                        [128, 384], F32, tag="psE", name="psE"
                    )

            def fc2_mm(ps2, tt, nch, h2, stop):
                for j in range(2):
                    ht = 2 * h2 + j
                    nc.tensor.matmul(
                        ps2[:],
                        gT[:, ht, tt * 128:(tt + 1) * 128],
                        w2_sb[:, ht, nch * 384:(nch + 1) * 384],
                        start=(ht == 0), stop=(stop and j == 1),
                        skip_group_check=True,
                    )

            for ht in range(HT):
                w1c = w1_p.tile([128, CT, 128], BF16, tag="w1c", name="w1c")
                nc.sync.dma_start(out=w1c[:], in_=w1r[:, ht])
                ps = ps_p.tile([128, 512], F32, tag="ps", name="ps")
                for kc in range(CT):
                    nc.tensor.matmul(
                        ps[:],
                        w1c[:, kc, :],
                        xn2T[:, kc, :],
                        start=(kc == 0), stop=(kc == CT - 1),
                    )
                nc.scalar.activation(out=gT[:, ht, :], in_=ps[:], func=AF.Gelu)
                if ht % 2 == 1:
                    for tt in range(NEARLY):
                        for nch in range(2):
                            fc2_mm(chains[(tt, nch)], tt, nch, ht // 2,
                                   stop=(ht == HT - 1))
            for tt in range(QT):
                outt = o_p.tile([128, C], F32, tag="outt", name="outt")
                for nch in range(2):
                    if tt < NEARLY:
                        ps2 = chains[(tt, nch)]
                    else:
                        ps2 = psE_p.tile([128, 384], F32, tag="psE", name="psE")
                        for h2 in range(HT // 2):
                            fc2_mm(ps2, tt, nch, h2, stop=(h2 == HT // 2 - 1))
                    nc.vector.scalar_tensor_tensor(
                        out=outt[:, nch * 384:(nch + 1) * 384],
                        in0=ps2[:], scalar=1.0,
                        in1=x1t[tt][:, nch * 384:(nch + 1) * 384],
                        op0=ALU.mult, op1=ALU.add,
                    )
                # spread output DMAs across queues so the final transfers
                # overlap instead of serializing on one queue
                eng = [nc.sync, nc.gpsimd, nc.scalar, nc.sync][tt]
                eng.dma_start(
                    out=out[tt * 128:(tt + 1) * 128, :], in_=outt[:]
                )


# ---------------------------------------------------------------- host side
_CACHED_NC = None


def _get_nc():
    global _CACHED_NC
    if _CACHED_NC is None:
        _CACHED_NC = build_program()
    return _CACHED_NC


def _part_major(a, inner_shape):
    """(CT*128, X) row-major -> (128, prod(inner_shape)) where the leading
    dim is split (blk, 128) and partitions become major: out[p, blk, :] =
    a[blk*128 + p, :]."""
    nblk = a.shape[0] // 128
    return np.ascontiguousarray(
        a.reshape((nblk, 128) + a.shape[1:]).swapaxes(0, 1).reshape(128, -1)
    )


def make_in_maps(x, mask, g1, b1, Wq, Wkv, Wp, bp, g2, b2, W1, bf1, W2, bf2):
    f32 = np.float32
    bf = ml_dtypes.bfloat16
    x = np.asarray(x, f32)
    mask = np.asarray(mask, f32)
    g1 = np.asarray(g1, f32); b1 = np.asarray(b1, f32)
    g2 = np.asarray(g2, f32); b2 = np.asarray(b2, f32)
    Wq = np.asarray(Wq, f32); Wkv = np.asarray(Wkv, f32); Wp = np.asarray(Wp, f32)
    W1 = np.asarray(W1, f32); W2 = np.asarray(W2, f32)
    bp = np.asarray(bp, f32); bf1 = np.asarray(bf1, f32); bf2 = np.asarray(bf2, f32)

    Wk, Wv = Wkv[:C], Wkv[C:]
    # fold LN gains + attention scale into the weights; biases must be zero
    # (they are, for this problem's setup_inputs) for this fast path.
    zero_rows = [
        (b1 @ Wq.T) * SCALE, b1 @ Wk.T, b1 @ Wv.T, bp,
        bf1 + b2 @ W1.T, bf2,
    ]
    for r in zero_rows:
        assert np.abs(r).max() == 0.0, "nonzero bias path not implemented"

    # device layouts ------------------------------------------------------
    f8 = ml_dtypes.float8_e4m3fn
    # wk/wq: [128, dt, kc, 128] with wk[p, dt, kc, q] = WkT[kc*128+p, dt*128+q]
    # QKV weights are uploaded fp8 at x8 scale (see XSCALE/WSCALE notes).
    wkT = (Wk * g1[None, :]).T * WSCALE   # (C in-feat, C out-feat)
    wqT = (Wq * g1[None, :] * SCALE).T * WSCALE
    wk_h = _part_major(wkT, None).reshape(128, CT, CT, 128)   # p, kc, dt, q
    wq_h = _part_major(wqT, None).reshape(128, CT, CT, 128)
    wk_h = np.ascontiguousarray(wk_h.swapaxes(1, 2)).reshape(128, -1).astype(f8)
    wq_h = np.ascontiguousarray(wq_h.swapaxes(1, 2)).reshape(128, -1).astype(f8)
    wv_h = _part_major((Wv * g1[None, :]).T * WSCALE, None).astype(f8)
    # v arrives at x(XSCALE*WSCALE) scale; fold the inverse into Wp
    wp_h = _part_major(Wp.T * PSCALE, None).astype(bf)
    # w1: [128, ht, kc, 128]: w1[p, ht, kc, q] = W1T[kc*128+p, ht*128+q]
    w1T = (W1 * g2[None, :]).T            # (C, HID)
    w1_h = _part_major(w1T, None).reshape(128, CT, HT, 128)
    w1_h = np.ascontiguousarray(w1_h.swapaxes(1, 2)).reshape(128, -1).astype(bf)
    w2_h = _part_major(W2.T, None).astype(bf)                  # p,(ht c)
    idn_h = np.eye(128, dtype=bf)

    # host-side LN1 (plain: gains/biases are folded into the weights above)
    mu = x.mean(axis=-1, keepdims=True)
    var = x.var(axis=-1, keepdims=True)
    xn_full = (x - mu) / np.sqrt(var + EPS)

    in_maps = []
    for c in range(NCORES):
        b, qi = divmod(c, 4)
        q0 = qi * QS
        xr = np.roll(x[b], -q0, axis=0)                    # my tokens first
        xnr = np.roll(xn_full[b], -q0, axis=0)
        # xnt: chunk-major feature-major: [128, tch, kc, 512]
        # xnt[p, tch, kc, t] = XSCALE * xn[tch*512 + t, kc*128 + p]
        xnt_h = np.ascontiguousarray(
            (xnr.T * XSCALE)
            .reshape(CT, 128, 4, QS).transpose(1, 2, 0, 3).reshape(128, -1)
        ).astype(f8)
        km = np.roll(1.0 - mask[b].T, -q0, axis=0)         # keys rolled too
        mmc = _part_major(
            np.ascontiguousarray(km[:, q0:q0 + QS]), None
        ).astype(bf)
        in_maps.append({
            "xnt": xnt_h,
            "xm": np.ascontiguousarray(xr[:QS]),
            "mm": mmc,
            "wqt": wq_h, "wkt": wk_h, "wvt": wv_h, "wpt": wp_h,
            "w1t": w1_h, "w2t": w2_h, "idn": idn_h,
        })
    return in_maps


def kernel(**inputs):
    nc = _get_nc()
    in_maps = make_in_maps(**inputs)
    res = run_bass_kernel_spmd(nc, in_maps, core_ids=list(range(NCORES)))
    out = np.empty((B, N, C), np.float32)
    for c in range(NCORES):
        b, qi = divmod(c, 4)
        q0 = qi * QS
        out[b, q0:q0 + QS] = res.results[c]["out"]
    return out


if __name__ == "__main__":
    print("building program...")
    nc = _get_nc()
    print("instructions:", sum(len(bb.instructions) for bb in nc.main_func.blocks))
